# revision 38
# baseline (speedup 1.0000x reference)
"""Trainium2 Bass kernel for a 2-layer GATv2 + JumpingKnowledge GNN.

Strategy (8 NeuronCores, dst-node sharding, 3 launches, zero on-device
gathers):
  - Host: add self loops, bucket edges by (core, 128-node dst window), pad
    every window to NSB 512-edge superblocks.  Build per-window one-hot
    matrices (node-major g01t and edge-major g01e) on the host.
  - Launch A (node-sharded): xl1 = x@Wl1, xr1 = x@Wr1 + bl1+br1,
    jk0 = x@Wjk0 for owned nodes.  Pure per-node GEMMs, ~40us.
  - Host: route xl1 rows into edge order (halo exchange): ship BOTH an
    edge-major copy (for the alpha-weighted message aggregation) and a
    feature-major copy (for the attention-logit pipeline).  Pure
    permutation of device-computed data - no FLOPs on host.
  - Launch B: layer-1 edge phase + h1 + layer-2 node transforms
    (xl2/xr2/jk01).
  - Host: route xl2 rows into edge order (same shapes as layer 1 / 2).
  - Launch C: layer-2 edge phase + JumpingKnowledge output projection.

Edge phase per 512-edge superblock (no gathers, no PE transposes):
  s_fm[g]   = xr_win[:,g] @ g01t  +  I @ xl_fm[g]        (PE, 2 matmuls/group)
  lr        = Prelu(s_fm, 0.2)                           (ACT)
  lg       += att_bd[g].T @ lr                           (PE)
  expf      = Exp(lg)                                    (ACT)
  expe      = transpose(expf) via 4 tiny PE matmuls      (PE)
  pr[b]     = xl_em[b] * expe[b]  (head-broadcast)       (DVE, 2x mode)
  U        += g01e[b].T @ pr[b] ; dn += g01e[b].T @ expe (PE, window accum)
Window epilogue: h = elu(U/dn + bias), then next-layer node GEMMs.

All feature axes use a head-interleaved order f=(c*H+h) so the DVE
broadcast multiply has innermost stride 1 (2x DVE perf mode).  Every
weight matrix is permuted accordingly on the host; the final output is
un-permuted (Wjk rows permuted to compensate).

The segment softmax skips the max subtraction: logits for this model are
in [-6, 6], exp() is safe, softmax is shift-invariant.
"""

import os
from contextlib import ExitStack

import ml_dtypes
import numpy as np

import concourse.bacc as bacc
import concourse.mybir as mybir
import concourse.tile as tile
from concourse.bass_utils import run_bass_kernel_spmd

dt = mybir.dt
AF = mybir.ActivationFunctionType
ALU = mybir.AluOpType
BF16 = ml_dtypes.bfloat16

# ---------------- problem constants (hardcoded per contract) ----------------
N = 20000
HID = 128
HEADS = 8
C1 = 64
C2 = 32
D1 = HEADS * C1  # 512
D2 = HEADS * C2  # 256

NCORES = 8
NPC = N // NCORES          # 2500 nodes per core
WNODES = 128               # nodes per window
NW = -(-NPC // WNODES)     # 20 windows per core
NPAD = NW * WNODES         # 2560 padded node slots per core
SBE = 512                  # edges per superblock

LAST_RESULTS = []          # BassKernelResults of the most recent kernel() call


def _bf(x):
    return np.ascontiguousarray(np.asarray(x, np.float32).astype(BF16))


def _f32(x):
    return np.ascontiguousarray(np.asarray(x, np.float32))


def _perm(D, H):
    """Head-interleave permutation: interleaved col j holds original col
    (j%H)*C + j//H  (i.e. j = c*H + h)."""
    j = np.arange(D)
    return (j % H) * (D // H) + j // H


PERM1 = _perm(D1, HEADS)
PERM2 = _perm(D2, HEADS)


def _att_bd(att, D):
    """[H, C] -> [128, nG*8] lhsT tiles of the interleaved block-diag."""
    H, C = att.shape
    nG = D // 128
    bd = np.zeros((D, H), np.float32)
    j = np.arange(D)
    bd[j, j % H] = att[j % H, j // H]
    return bd.reshape(nG, 128, H).transpose(1, 0, 2).reshape(128, nG * 8)


def _plan_edges(edge_index):
    """Bucket self-loop-augmented edges by (core, window); pad to NSB
    superblocks of SBE edges.  Returns (NSB, srcs, goh) where
      srcs[c][w] = int64 src node per padded edge slot (0 for pads)
      goh[c]     = [NW, 128, 2*EPW] bf16  (g01t || g01e one-hots)"""
    src = np.concatenate([edge_index[0].astype(np.int64),
                          np.arange(N, dtype=np.int64)])
    dst = np.concatenate([edge_index[1].astype(np.int64),
                          np.arange(N, dtype=np.int64)])
    core = dst // NPC
    dloc = dst - core * NPC
    win = dloc // WNODES
    din = dloc % WNODES

    order = np.lexsort((win, core))
    src, core, win, din = src[order], core[order], win[order], din[order]

    lists = {}
    nsb = 1
    for c in range(NCORES):
        mc = core == c
        sc, wc, dc = src[mc], win[mc], din[mc]
        for w in range(NW):
            mw = wc == w
            lists[(c, w)] = (sc[mw], dc[mw])
            nsb = max(nsb, -(-int(mw.sum()) // SBE))
    epw = nsb * SBE

    srcs, gohs = [], []
    e = np.arange(epw)
    blk, pin = e // 128, e % 128
    for c in range(NCORES):
        sp_all = np.zeros((NW, epw), np.int64)
        goh = np.zeros((NW, 128, 2 * epw), np.float32)
        for w in range(NW):
            s_, d_ = lists[(c, w)]
            ne = len(s_)
            sp_all[w, :ne] = s_
            # g01t[n, e] = (din[e] == n)
            goh[w, d_, np.arange(ne)] = 1.0
            # g01e[e%128, epw + (e//128)*128 + n] = (din[e] == n)
            goh[w, pin[:ne], epw + blk[:ne] * 128 + d_] = 1.0
        srcs.append(sp_all)
        gohs.append(_bf(goh))
    return nsb, srcs, gohs


def _route_edges(table_bf, srcs, nsb, with_ones):
    """Gather table rows into edge order, per core: em||fm per superblock.

    table_bf: [N, D] bf16 (feature cols already head-interleaved)
    returns list of [NW, 128, NSB*SBSZ] bf16 arrays; per-sb slice is
      em [128, 4, DE]  (with_ones: DE=D+8, last 8 cols per block are 1.0 -
                        they carry the softmax denominator through the U
                        matmul; only legal when DE <= 512)
      || fm [128, D//128, 512]"""
    D = table_bf.shape[1]
    DE = D + 8 if with_ones else D
    nG = D // 128
    sbsz = 4 * DE + nG * SBE
    out = []
    for sp_all in srcs:
        gat = table_bf[sp_all.reshape(-1)].reshape(NW, nsb, SBE, D)
        if with_ones:
            gata = np.empty((NW, nsb, SBE, DE), BF16)
            gata[..., :D] = gat
            gata[..., D:] = np.float32(1.0)
        else:
            gata = gat
        # em[p, b, f] = gata[sb, b*128+p, f]
        em = gata.reshape(NW, nsb, 4, 128, DE).transpose(0, 1, 3, 2, 4)
        em = np.ascontiguousarray(em).reshape(NW, nsb, 128, 4 * DE)
        # fm[p, g, e] = gat[sb, e, g*128+p]
        fm = gat.transpose(0, 1, 3, 2).reshape(NW, nsb, nG, 128, SBE)
        fm = np.ascontiguousarray(fm.transpose(0, 1, 3, 2, 4))
        fm = fm.reshape(NW, nsb, 128, nG * SBE)
        both = np.concatenate([em, fm], axis=3)       # [NW, nsb, 128, sbsz]
        both = np.ascontiguousarray(both.transpose(0, 2, 1, 3))
        out.append(both.reshape(NW, 128, nsb * sbsz))
    return out


# ------------------------------ launch A -----------------------------------

def _build_launch_a():
    nc = bacc.Bacc(None, target_bir_lowering=False)
    x_ownT = nc.dram_tensor("x_ownT", [128, NPAD], dt.bfloat16,
                            kind="ExternalInput")
    Wl1p = nc.dram_tensor("Wl1p", [128, D1], dt.bfloat16, kind="ExternalInput")
    Wr1p = nc.dram_tensor("Wr1p", [128, D1], dt.bfloat16, kind="ExternalInput")
    bxr1p = nc.dram_tensor("bxr1p", [128, D1], dt.float32, kind="ExternalInput")
    Wjk0 = nc.dram_tensor("Wjk0", [128, 128], dt.bfloat16, kind="ExternalInput")

    ao = nc.dram_tensor("ao", [NPAD, 2 * D1 + 128], dt.bfloat16,
                        kind="ExternalOutput")

    with tile.TileContext(nc) as tc, ExitStack() as ctx:
        const = ctx.enter_context(tc.tile_pool(name="const", bufs=1))
        sbuf = ctx.enter_context(tc.tile_pool(name="sbuf", bufs=3))
        pp = ctx.enter_context(tc.tile_pool(name="pp", bufs=4, space="PSUM"))
        pps = ctx.enter_context(tc.tile_pool(name="pps", bufs=2, space="PSUM"))

        def cl(name, hdl, shape, dtype):
            t = const.tile(shape, dtype, tag=name)
            nc.sync.dma_start(t[:], hdl[:])
            return t

        wl = cl("wl", Wl1p, [128, D1], dt.bfloat16)
        wr = cl("wr", Wr1p, [128, D1], dt.bfloat16)
        bx = cl("bx", bxr1p, [128, D1], dt.float32)
        wj = cl("wj", Wjk0, [128, 128], dt.bfloat16)
        xo = const.tile([128, NPAD], dt.bfloat16, tag="xo")
        nc.sync.dma_start(xo[:], x_ownT[:])

        outq = [nc.gpsimd, nc.scalar, nc.sync]
        for w in range(NW):
            lhs = xo[:, w * 128:(w + 1) * 128]
            t1 = sbuf.tile([128, 2 * D1 + 128], dt.bfloat16, tag="t1")
            p1 = pp.tile([128, D1], dt.float32, tag="p1")
            nc.tensor.matmul(p1[:], lhsT=lhs, rhs=wl[:], start=True, stop=True)
            nc.vector.tensor_copy(t1[:, :D1], p1[:])
            p2 = pp.tile([128, D1], dt.float32, tag="p1")
            nc.tensor.matmul(p2[:], lhsT=lhs, rhs=wr[:], start=True, stop=True)
            nc.vector.tensor_tensor(out=t1[:, D1:2 * D1], in0=p2[:], in1=bx[:],
                                    op=ALU.add)
            p3 = pps.tile([128, 128], dt.float32, tag="p3")
            nc.tensor.matmul(p3[:], lhsT=lhs, rhs=wj[:], start=True, stop=True)
            nc.scalar.activation(t1[:, 2 * D1:], p3[:], AF.Copy)
            outq[w % 3].dma_start(ao[w * 128:(w + 1) * 128, :], t1[:])

    nc.compile()
    return nc


# ------------------------- edge-phase launches ------------------------------

def _emit_edge_pipeline(nc, pools, cfg):
    """Software-pipelined edge phase + window epilogues for one GAT layer.

    Pipeline stages (each lags the previous by one superblock iteration):
      phase1(k):  ef DMA, s matmuls (xr scatter + fm accum), Prelu, lg, Exp
      phase2a(k): ept transpose minis, expe copy, pr = em*expe (DVE 2x)
      phase2b(k): U += g01e.T @ pr  (also accumulates the denominator via
                  the ones columns baked into em)
    epi_v(w) is emitted right after phase2b(w, NSB-1); on_h(w) two
    iterations later so the PE never waits on the DVE elu chain."""
    sbuf, empool, gohpool = pools["sbuf"], pools["em"], pools["goh"]
    ppS, ppLG, ppE, ppU = (pools["ppS"], pools["ppLG"], pools["ppE"],
                           pools["ppU"])
    ppDN = pools.get("ppDN")
    D, CH, NSB = cfg["D"], cfg["CH"], cfg["NSB"]
    merged = cfg["merged_dn"]         # denominator rides in U's ones columns
    DE = D + 8 if merged else D
    nG = D // 128
    EPW = NSB * SBE
    SBSZ = 4 * DE + nG * SBE  # per-sb free elements: em (4*DE) || fm (nG*SBE)
    ident = cfg["ident"]

    state = {}

    def phase1(w, sb, ef):
        goh_t = state[("goh", w)]
        lg = ppLG.tile([8, SBE], dt.float32, tag="lg")
        ss, lrs = [], []
        for g in range(nG):
            s = ppS.tile([128, SBE], dt.float32, tag="s")
            nc.tensor.matmul(
                s[:], lhsT=cfg["xr_tile"][:, w * D + g * 128:w * D + (g + 1) * 128],
                rhs=goh_t[:, sb * SBE:(sb + 1) * SBE], start=True, stop=False)
            nc.tensor.matmul(
                s[:], lhsT=ident[:],
                rhs=ef[:, 4 * DE + g * SBE:4 * DE + (g + 1) * SBE],
                start=False, stop=True)
            lr = sbuf.tile([128, SBE], dt.bfloat16, tag="lr")
            nc.scalar.activation(lr[:], s[:], AF.Prelu, alpha=0.2)
            ss.append(s)
            lrs.append(lr)
            # lag the lg matmul one group behind the s matmuls so the PE
            # never waits on the Prelu
            if g >= 1:
                nc.tensor.matmul(lg[:],
                                 lhsT=cfg["att_tile"][:, (g - 1) * 8:g * 8],
                                 rhs=lrs[g - 1][:], start=(g == 1), stop=False)
        nc.tensor.matmul(lg[:], lhsT=cfg["att_tile"][:, (nG - 1) * 8:nG * 8],
                         rhs=lrs[nG - 1][:], start=(nG == 1), stop=True)
        expf = sbuf.tile([8, SBE], dt.bfloat16, tag="expf")
        nc.scalar.activation(expf[:], lg[:], AF.Exp)
        return ef, expf

    def phase2a(w, sb, ef, expf):
        ept = ppE.tile([128, 32], dt.float32, tag="ept")
        for b in range(4):
            nc.tensor.matmul(ept[:, b * 8:(b + 1) * 8],
                             lhsT=expf[:, b * 128:(b + 1) * 128],
                             rhs=ident[:8, :8],
                             start=(b == 0), stop=(b == 3))
        expe = sbuf.tile([128, 32], dt.bfloat16, tag="expe")
        nc.scalar.activation(expe[:], ept[:], AF.Copy)
        CHE = CH + 1 if merged else CH
        pr = sbuf.tile([128, 4 * DE], dt.bfloat16, tag="pr")
        for b in range(4):
            nc.vector.tensor_tensor(
                out=pr[:, b * DE:(b + 1) * DE]
                    .rearrange("p (c h) -> p c h", h=8),
                in0=ef[:, b * DE:(b + 1) * DE]
                    .rearrange("p (c h) -> p c h", h=8),
                in1=expe[:, b * 8:(b + 1) * 8].unsqueeze(1)
                    .broadcast_to([128, CHE, 8]),
                op=ALU.mult)
        return pr, expe

    def phase2b(w, sb, pr, expe):
        goh_t = state[("goh", w)]
        U, dn = state[("U", w)]
        for b in range(4):
            lh = goh_t[:, EPW + (sb * 4 + b) * 128:EPW + (sb * 4 + b + 1) * 128]
            first = (sb == 0 and b == 0)
            last = (sb == NSB - 1 and b == 3)
            nc.tensor.matmul(U[:], lhsT=lh, rhs=pr[:, b * DE:(b + 1) * DE],
                             start=first, stop=last)
            if not merged:
                nc.tensor.matmul(dn[:], lhsT=lh,
                                 rhs=expe[:, b * 8:(b + 1) * 8],
                                 start=first, stop=last)

    def epi_v(w):
        U, dn = state[("U", w)]
        dns = sbuf.tile([128, 8], dt.float32, tag="dns")
        nc.vector.tensor_scalar_max(dns[:], U[:, D:DE] if merged else dn[:],
                                    1e-30)
        rd = sbuf.tile([128, 8], dt.float32, tag="rd")
        nc.vector.reciprocal(rd[:], dns[:])
        v = sbuf.tile([128, D], dt.float32, tag="v")
        nc.vector.tensor_tensor(
            out=v[:].rearrange("p (c h) -> p c h", h=8),
            in0=(U[:, :D] if merged else U[:])
                .rearrange("p (c h) -> p c h", h=8),
            in1=rd[:].unsqueeze(1).broadcast_to([128, CH, 8]),
            op=ALU.mult)
        # elu(vb) = max(vb, exp(min(vb, 0)) - 1); bias-add and clamp run on
        # the otherwise-idle gpsimd engine so the DVE queue stays short
        vb = sbuf.tile([128, D], dt.float32, tag="vb")
        nc.gpsimd.tensor_tensor(out=vb[:], in0=v[:], in1=cfg["biash_tile"][:],
                                op=ALU.add)
        m = sbuf.tile([128, D], dt.float32, tag="m")
        nc.gpsimd.tensor_scalar_min(m[:], vb[:], 0.0)
        em_ = sbuf.tile([128, D], dt.float32, tag="em_")
        nc.scalar.activation(em_[:], m[:], AF.Exp)
        h = sbuf.tile([128, D], dt.bfloat16, tag="h")
        nc.vector.scalar_tensor_tensor(out=h[:], in0=em_[:], scalar=-1.0,
                                       op0=ALU.add, in1=vb[:], op1=ALU.max)
        state[("h", w)] = h

    ef_q = []

    def prefetch(w, sb):
        # issued one full iteration ahead of use so DMA transfers are hidden
        if sb == 0:
            goh_t = gohpool.tile([128, 2 * EPW], dt.bfloat16, tag="goh")
            nc.gpsimd.dma_start(goh_t[:], cfg["goh_dram"][w])
            state[("goh", w)] = goh_t
            if cfg.get("load_xr"):
                cfg["load_xr"](w)
        ef = empool.tile([128, SBSZ], dt.bfloat16, tag="ef")
        nc.gpsimd.dma_start(
            ef[:], cfg["emfm_dram"][w][:, sb * SBSZ:(sb + 1) * SBSZ])
        ef_q.append(ef)

    def begin_window(w):
        U = ppU.tile([128, DE], dt.float32, tag="U")
        dn = None
        if not merged:
            dn = ppDN.tile([128, 8], dt.float32, tag="dn")
        state[("U", w)] = (U, dn)

    items = [(w, sb) for w in range(NW) for sb in range(NSB)]
    fifo_a, fifo_b = [], []   # pending phase2a / phase2b work
    epi_cd = []               # [w, countdown] until on_h emission
    prefetch(*items[0])

    def tick():
        if len(fifo_b) > 1:
            w, sb, pr, expe = fifo_b.pop(0)
            phase2b(w, sb, pr, expe)
            if sb == NSB - 1:
                epi_v(w)
                epi_cd.append([w, 2])
        for e in epi_cd:
            e[1] -= 1
        while epi_cd and epi_cd[0][1] <= 0:
            w = epi_cd.pop(0)[0]
            cfg["on_h"](w, state.pop(("h", w)))
            del state[("goh", w)], state[("U", w)]

    for i, (w, sb) in enumerate(items):
        if sb == 0:
            begin_window(w)
        if i + 1 < len(items):
            prefetch(*items[i + 1])
        ef, expf = phase1(w, sb, ef_q.pop(0))
        if i == 0 and cfg.get("late_consts"):
            cfg["late_consts"]()
        if len(fifo_a) > 0:
            pw, psb, pef, pexpf = fifo_a.pop(0)
            pr, expe = phase2a(pw, psb, pef, pexpf)
            fifo_b.append((pw, psb, pr, expe))
        fifo_a.append((w, sb, ef, expf))
        tick()
    # drain
    while fifo_a:
        pw, psb, pef, pexpf = fifo_a.pop(0)
        pr, expe = phase2a(pw, psb, pef, pexpf)
        fifo_b.append((pw, psb, pr, expe))
    while fifo_b:
        w, sb, pr, expe = fifo_b.pop(0)
        phase2b(w, sb, pr, expe)
        if sb == NSB - 1:
            epi_v(w)
            epi_cd.append([w, 0])
    while epi_cd:
        w = epi_cd.pop(0)[0]
        cfg["on_h"](w, state.pop(("h", w)))
        del state[("goh", w)], state[("U", w)]


def _build_launch_b(NSB):
    EPW = NSB * SBE
    nc = bacc.Bacc(None, target_bir_lowering=False)

    emfm = nc.dram_tensor("emfm", [NW, 128, NSB * (4 * D1 + 4 * SBE)],
                          dt.bfloat16, kind="ExternalInput")
    goh = nc.dram_tensor("goh", [NW, 128, 2 * EPW], dt.bfloat16,
                         kind="ExternalInput")
    xr1 = nc.dram_tensor("xr1", [NW, 128, D1], dt.bfloat16,
                         kind="ExternalInput")
    jk0 = nc.dram_tensor("jk0", [NW, 128, 128], dt.bfloat16,
                         kind="ExternalInput")
    att1bd = nc.dram_tensor("att1bd", [128, 32], dt.bfloat16,
                            kind="ExternalInput")
    biash1 = nc.dram_tensor("biash1", [128, D1], dt.float32,
                            kind="ExternalInput")
    identI = nc.dram_tensor("identI", [128, 128], dt.bfloat16,
                            kind="ExternalInput")
    Wl2p = nc.dram_tensor("Wl2p", [128, 4 * D2], dt.bfloat16,
                          kind="ExternalInput")
    Wr2p = nc.dram_tensor("Wr2p", [128, 4 * D2], dt.bfloat16,
                          kind="ExternalInput")
    bxr2p = nc.dram_tensor("bxr2p", [128, D2], dt.float32,
                           kind="ExternalInput")
    Wjk1p = nc.dram_tensor("Wjk1p", [128, 4 * 128], dt.bfloat16,
                           kind="ExternalInput")

    xl2_o = nc.dram_tensor("xl2_o", [NPAD, D2], dt.bfloat16,
                           kind="ExternalOutput")
    xr2_o = nc.dram_tensor("xr2_o", [NPAD, D2], dt.bfloat16,
                           kind="ExternalOutput")
    jk01_o = nc.dram_tensor("jk01_o", [NPAD, 128], dt.float32,
                            kind="ExternalOutput")

    with tile.TileContext(nc) as tc, ExitStack() as ctx:
        const = ctx.enter_context(tc.tile_pool(name="const", bufs=1))
        sbuf = ctx.enter_context(tc.tile_pool(name="sbuf", bufs=3))
        empool = ctx.enter_context(tc.tile_pool(name="em", bufs=3))
        gohpool = ctx.enter_context(tc.tile_pool(name="goh", bufs=2))
        ppS = ctx.enter_context(tc.tile_pool(name="ppS", bufs=2, space="PSUM"))
        ppLG = ctx.enter_context(tc.tile_pool(name="ppLG", bufs=1, space="PSUM"))
        ppE = ctx.enter_context(tc.tile_pool(name="ppE", bufs=1, space="PSUM"))
        ppU = ctx.enter_context(tc.tile_pool(name="ppU", bufs=2, space="PSUM"))
        ppDN = ctx.enter_context(tc.tile_pool(name="ppDN", bufs=2, space="PSUM"))

        def cl(name, hdl, shape, dtype):
            t = const.tile(shape, dtype, tag=name)
            nc.sync.dma_start(t[:], hdl[:])
            return t

        ident = cl("ident", identI, [128, 128], dt.bfloat16)
        att1_t = cl("att1", att1bd, [128, 32], dt.bfloat16)
        bh1_t = const.tile([128, D1], dt.float32, tag="bh1")
        wl2_t = const.tile([128, 4 * D2], dt.bfloat16, tag="wl2")
        wr2_t = const.tile([128, 4 * D2], dt.bfloat16, tag="wr2")
        bxr2_t = const.tile([128, D2], dt.float32, tag="bxr2")
        wjk1_t = const.tile([128, 4 * 128], dt.bfloat16, tag="wjk1")
        xr1_t = const.tile([128, NW * D1], dt.bfloat16, tag="xr1t")

        def late_consts():
            nc.sync.dma_start(bh1_t[:], biash1[:])
            nc.sync.dma_start(wl2_t[:], Wl2p[:])
            nc.sync.dma_start(wr2_t[:], Wr2p[:])
            nc.sync.dma_start(bxr2_t[:], bxr2p[:])
            nc.sync.dma_start(wjk1_t[:], Wjk1p[:])

        def load_xr(w):
            nc.sync.dma_start(xr1_t[:, w * D1:(w + 1) * D1], xr1[w])

        def on_h(w, h):
            # xl2 = h@Wl2p ; xr2 = h@Wr2p + b ; jk01 = jk0 + h@Wjk1p
            p_xl2 = ppS.tile([128, D2], dt.float32, tag="s")
            p_xr2 = ppS.tile([128, D2], dt.float32, tag="s")
            p_jk = ppE.tile([128, 128], dt.float32, tag="ept")
            for g in range(4):
                tp = ppLG.tile([128, 128], dt.float32, tag="lg")
                nc.tensor.matmul(tp[:], lhsT=h[:, g * 128:(g + 1) * 128],
                                 rhs=ident[:], start=True, stop=True)
                hTs = sbuf.tile([128, 128], dt.bfloat16, tag="hT")
                nc.vector.tensor_copy(hTs[:], tp[:])
                nc.tensor.matmul(p_xl2[:], lhsT=hTs[:],
                                 rhs=wl2_t[:, g * D2:(g + 1) * D2],
                                 start=(g == 0), stop=(g == 3))
                nc.tensor.matmul(p_xr2[:], lhsT=hTs[:],
                                 rhs=wr2_t[:, g * D2:(g + 1) * D2],
                                 start=(g == 0), stop=(g == 3))
                nc.tensor.matmul(p_jk[:], lhsT=hTs[:],
                                 rhs=wjk1_t[:, g * 128:(g + 1) * 128],
                                 start=(g == 0), stop=(g == 3))
            o_xl2 = sbuf.tile([128, D2], dt.bfloat16, tag="oxl2")
            nc.any.tensor_copy(o_xl2[:], p_xl2[:])
            nc.gpsimd.dma_start(xl2_o[w * 128:(w + 1) * 128, :], o_xl2[:])
            o_xr2 = sbuf.tile([128, D2], dt.bfloat16, tag="oxr2")
            nc.vector.tensor_tensor(out=o_xr2[:], in0=p_xr2[:], in1=bxr2_t[:],
                                    op=ALU.add)
            nc.gpsimd.dma_start(xr2_o[w * 128:(w + 1) * 128, :], o_xr2[:])
            jk0_t = sbuf.tile([128, 128], dt.bfloat16, tag="jk0")
            nc.gpsimd.dma_start(jk0_t[:], jk0[w])
            o_jk = sbuf.tile([128, 128], dt.float32, tag="ojk")
            nc.vector.tensor_tensor(out=o_jk[:], in0=p_jk[:], in1=jk0_t[:],
                                    op=ALU.add)
            nc.gpsimd.dma_start(jk01_o[w * 128:(w + 1) * 128, :], o_jk[:])

        pools = dict(sbuf=sbuf, em=empool, goh=gohpool, ppS=ppS, ppLG=ppLG,
                     ppE=ppE, ppU=ppU, ppDN=ppDN)
        _emit_edge_pipeline(nc, pools, dict(
            D=D1, CH=C1, NSB=NSB, merged_dn=False,
            emfm_dram=emfm, goh_dram=goh,
            xr_tile=xr1_t, att_tile=att1_t, biash_tile=bh1_t,
            ident=ident, on_h=on_h, late_consts=late_consts, load_xr=load_xr))

    nc.compile()
    return nc


def _build_launch_c(NSB):
    EPW = NSB * SBE
    nc = bacc.Bacc(None, target_bir_lowering=False)

    emfm = nc.dram_tensor("emfm", [NW, 128, NSB * (4 * (D2 + 8) + 2 * SBE)],
                          dt.bfloat16, kind="ExternalInput")
    goh = nc.dram_tensor("goh", [NW, 128, 2 * EPW], dt.bfloat16,
                         kind="ExternalInput")
    xr2 = nc.dram_tensor("xr2", [NW, 128, D2], dt.bfloat16,
                         kind="ExternalInput")
    jk01 = nc.dram_tensor("jk01", [NW, 128, 128], dt.float32,
                          kind="ExternalInput")
    att2bd = nc.dram_tensor("att2bd", [128, 16], dt.bfloat16,
                            kind="ExternalInput")
    biash2 = nc.dram_tensor("biash2", [128, D2], dt.float32,
                            kind="ExternalInput")
    identI = nc.dram_tensor("identI", [128, 128], dt.bfloat16,
                            kind="ExternalInput")
    Wjk2p = nc.dram_tensor("Wjk2p", [128, 2 * 128], dt.bfloat16,
                           kind="ExternalInput")
    bjk_r = nc.dram_tensor("bjk_r", [1, 128], dt.bfloat16,
                           kind="ExternalInput")
    ones1d = nc.dram_tensor("ones1", [1, 128], dt.bfloat16,
                            kind="ExternalInput")

    out_o = nc.dram_tensor("out_o", [NPAD, 128], dt.float32,
                           kind="ExternalOutput")

    with tile.TileContext(nc) as tc, ExitStack() as ctx:
        const = ctx.enter_context(tc.tile_pool(name="const", bufs=1))
        sbuf = ctx.enter_context(tc.tile_pool(name="sbuf", bufs=3))
        empool = ctx.enter_context(tc.tile_pool(name="em", bufs=3))
        gohpool = ctx.enter_context(tc.tile_pool(name="goh", bufs=2))
        ppS = ctx.enter_context(tc.tile_pool(name="ppS", bufs=2, space="PSUM"))
        ppLG = ctx.enter_context(tc.tile_pool(name="ppLG", bufs=1, space="PSUM"))
        ppE = ctx.enter_context(tc.tile_pool(name="ppE", bufs=1, space="PSUM"))
        ppU = ctx.enter_context(tc.tile_pool(name="ppU", bufs=2, space="PSUM"))

        def cl(name, hdl, shape, dtype):
            t = const.tile(shape, dtype, tag=name)
            nc.sync.dma_start(t[:], hdl[:])
            return t

        ident = cl("ident", identI, [128, 128], dt.bfloat16)
        att2_t = cl("att2", att2bd, [128, 16], dt.bfloat16)
        bh2_t = const.tile([128, D2], dt.float32, tag="bh2")
        wjk2_t = const.tile([128, 2 * 128], dt.bfloat16, tag="wjk2")
        bjkr_t = cl("bjkr", bjk_r, [1, 128], dt.bfloat16)
        ones1 = cl("ones1", ones1d, [1, 128], dt.bfloat16)
        xr2_t = const.tile([128, NW * D2], dt.bfloat16, tag="xr2t")

        def late_consts():
            nc.sync.dma_start(bh2_t[:], biash2[:])
            nc.sync.dma_start(wjk2_t[:], Wjk2p[:])

        def load_xr(w):
            nc.sync.dma_start(xr2_t[:, w * D2:(w + 1) * D2], xr2[w])

        def on_h(w, h):
            p_out = ppS.tile([128, 128], dt.float32, tag="s")
            nc.tensor.matmul(p_out[:], lhsT=ones1[:], rhs=bjkr_t[:],
                             start=True, stop=False)
            for g in range(2):
                tp = ppLG.tile([128, 128], dt.float32, tag="lg")
                nc.tensor.matmul(tp[:], lhsT=h[:, g * 128:(g + 1) * 128],
                                 rhs=ident[:], start=True, stop=True)
                hTs = sbuf.tile([128, 128], dt.bfloat16, tag="hT")
                nc.vector.tensor_copy(hTs[:], tp[:])
                nc.tensor.matmul(p_out[:], lhsT=hTs[:],
                                 rhs=wjk2_t[:, g * 128:(g + 1) * 128],
                                 start=False, stop=(g == 1))
            jk_t = sbuf.tile([128, 128], dt.float32, tag="jkt")
            nc.gpsimd.dma_start(jk_t[:], jk01[w])
            o_t = sbuf.tile([128, 128], dt.float32, tag="ot")
            nc.vector.tensor_tensor(out=o_t[:], in0=p_out[:], in1=jk_t[:],
                                    op=ALU.add)
            nc.gpsimd.dma_start(out_o[w * 128:(w + 1) * 128, :], o_t[:])

        pools = dict(sbuf=sbuf, em=empool, goh=gohpool, ppS=ppS, ppLG=ppLG,
                     ppE=ppE, ppU=ppU)
        _emit_edge_pipeline(nc, pools, dict(
            D=D2, CH=C2, NSB=NSB, merged_dn=True,
            emfm_dram=emfm, goh_dram=goh,
            xr_tile=xr2_t, att_tile=att2_t, biash_tile=bh2_t,
            ident=ident, on_h=on_h, late_consts=late_consts, load_xr=load_xr))

    nc.compile()
    return nc


_PROGRAM_CACHE = {}


def kernel(x, edge_index, Wl1, bl1, Wr1, br1, att1, bias1,
           Wl2, bl2, Wr2, br2, att2, bias2, Wjk, bjk):
    global LAST_RESULTS
    LAST_RESULTS = []
    trace = bool(os.environ.get("GAT_TRACE"))

    x = _f32(x)
    Wl1, Wr1 = _f32(Wl1), _f32(Wr1)
    Wl2, Wr2 = _f32(Wl2), _f32(Wr2)
    Wjk = _f32(Wjk)
    NSB, srcs, gohs = _plan_edges(np.asarray(edge_index))

    if "A" not in _PROGRAM_CACHE:
        _PROGRAM_CACHE["A"] = _build_launch_a()
    if ("B", NSB) not in _PROGRAM_CACHE:
        _PROGRAM_CACHE[("B", NSB)] = _build_launch_b(NSB)
    if ("C", NSB) not in _PROGRAM_CACHE:
        _PROGRAM_CACHE[("C", NSB)] = _build_launch_c(NSB)

    ident = np.eye(128, dtype=np.float32)

    # ---------------- launch A: per-node transforms ----------------
    common_a = dict(
        Wl1p=_bf(Wl1[:, PERM1]),
        Wr1p=_bf(Wr1[:, PERM1]),
        bxr1p=_f32(np.tile((np.asarray(bl1) + np.asarray(br1))[PERM1][None, :],
                           (128, 1))),
        Wjk0=_bf(Wjk[:128]),
    )
    in_maps_a = []
    for c in range(NCORES):
        xo = np.zeros((128, NPAD), np.float32)
        xo[:, :NPC] = x[c * NPC:(c + 1) * NPC].T
        in_maps_a.append(dict(common_a, x_ownT=_bf(xo)))

    res_a = run_bass_kernel_spmd(_PROGRAM_CACHE["A"], in_maps_a,
                                 core_ids=list(range(NCORES)), trace=trace)
    LAST_RESULTS.append(res_a)

    # ---------------- host routing for layer 1 ----------------
    ao = [np.asarray(res_a.results[c]["ao"]) for c in range(NCORES)]
    xl1_all = np.concatenate([a[:NPC, :D1] for a in ao], axis=0)
    emfm1 = _route_edges(xl1_all, srcs, NSB, with_ones=False)

    common_b = dict(
        att1bd=_bf(_att_bd(np.asarray(att1), D1)),
        biash1=_f32(np.tile((np.asarray(bl1) + np.asarray(bias1))[PERM1][None, :],
                            (128, 1))),
        identI=_bf(ident),
        Wl2p=_bf(Wl2[PERM1][:, PERM2].reshape(4, 128, D2)
                 .transpose(1, 0, 2).reshape(128, 4 * D2)),
        Wr2p=_bf(Wr2[PERM1][:, PERM2].reshape(4, 128, D2)
                 .transpose(1, 0, 2).reshape(128, 4 * D2)),
        bxr2p=_f32(np.tile((np.asarray(bl2) + np.asarray(br2))[PERM2][None, :],
                           (128, 1))),
        Wjk1p=_bf(Wjk[128:128 + D1][PERM1].reshape(4, 128, 128)
                  .transpose(1, 0, 2).reshape(128, 4 * 128)),
    )
    in_maps_b = []
    for c in range(NCORES):
        in_maps_b.append(dict(
            common_b,
            emfm=emfm1[c],
            goh=gohs[c],
            xr1=np.ascontiguousarray(ao[c][:, D1:2 * D1].reshape(NW, 128, D1)),
            jk0=np.ascontiguousarray(ao[c][:, 2 * D1:].reshape(NW, 128, 128)),
        ))

    res_b = run_bass_kernel_spmd(_PROGRAM_CACHE[("B", NSB)], in_maps_b,
                                 core_ids=list(range(NCORES)), trace=trace)
    LAST_RESULTS.append(res_b)

    # ---------------- host routing for layer 2 ----------------
    xl2_all = np.concatenate(
        [np.asarray(res_b.results[c]["xl2_o"])[:NPC] for c in range(NCORES)],
        axis=0)                                   # [N, 256] bf16, interleaved
    emfm2 = _route_edges(xl2_all, srcs, NSB, with_ones=True)

    common_c = dict(
        att2bd=_bf(_att_bd(np.asarray(att2), D2)),
        biash2=_f32(np.tile((np.asarray(bl2) + np.asarray(bias2))[PERM2][None, :],
                            (128, 1))),
        identI=_bf(ident),
        Wjk2p=_bf(Wjk[128 + D1:][PERM2].reshape(2, 128, 128)
                  .transpose(1, 0, 2).reshape(128, 2 * 128)),
        bjk_r=_bf(np.asarray(bjk)[None, :]),
        ones1=_bf(np.ones((1, 128), np.float32)),
    )
    in_maps_c = []
    for c in range(NCORES):
        in_maps_c.append(dict(
            common_c,
            emfm=emfm2[c],
            goh=gohs[c],
            xr2=np.asarray(res_b.results[c]["xr2_o"]).reshape(NW, 128, D2),
            jk01=_f32(np.asarray(res_b.results[c]["jk01_o"])
                      .reshape(NW, 128, 128)),
        ))

    res_c = run_bass_kernel_spmd(_PROGRAM_CACHE[("C", NSB)], in_maps_c,
                                 core_ids=list(range(NCORES)), trace=trace)
    LAST_RESULTS.append(res_c)

    out = np.concatenate(
        [np.asarray(res_c.results[c]["out_o"])[:NPC] for c in range(NCORES)],
        axis=0)
    return np.ascontiguousarray(out, dtype=np.float32)


# revision 41
# speedup vs baseline: 1.1340x; 1.1340x over previous
"""Trainium2 Bass kernel for a 2-layer GATv2 + JumpingKnowledge GNN.

Strategy (8 NeuronCores, dst-node sharding, 3 launches, zero on-device
gathers):
  - Host: add self loops, bucket edges by (core, 128-node dst window), pad
    every window to NSB 512-edge superblocks.  Build per-window one-hot
    matrices (node-major g01t and edge-major g01e) on the host.
  - Launch A (node-sharded): xl1 = x@Wl1, xr1 = x@Wr1 + bl1+br1,
    jk0 = x@Wjk0 for owned nodes.  Pure per-node GEMMs, ~40us.
  - Host: route xl1 rows into edge order (halo exchange): ship BOTH an
    edge-major copy (for the alpha-weighted message aggregation) and a
    feature-major copy (for the attention-logit pipeline).  Pure
    permutation of device-computed data - no FLOPs on host.
  - Launch B: layer-1 edge phase + h1 + layer-2 node transforms
    (xl2/xr2/jk01).
  - Host: route xl2 rows into edge order (same shapes as layer 1 / 2).
  - Launch C: layer-2 edge phase + JumpingKnowledge output projection.

Edge phase per 512-edge superblock (no gathers, no PE transposes):
  s_fm[g]   = xr_win[:,g] @ g01t  +  I @ xl_fm[g]        (PE, 2 matmuls/group)
  lr        = Prelu(s_fm, 0.2)                           (ACT)
  lg       += att_bd[g].T @ lr                           (PE)
  expf      = Exp(lg)                                    (ACT)
  expe      = transpose(expf) via 4 tiny PE matmuls      (PE)
  pr[b]     = xl_em[b] * expe[b]  (head-broadcast)       (DVE, 2x mode)
  U        += g01e[b].T @ pr[b] ; dn += g01e[b].T @ expe (PE, window accum)
Window epilogue: h = elu(U/dn + bias), then next-layer node GEMMs.

All feature axes use a head-interleaved order f=(c*H+h) so the DVE
broadcast multiply has innermost stride 1 (2x DVE perf mode).  Every
weight matrix is permuted accordingly on the host; the final output is
un-permuted (Wjk rows permuted to compensate).

The segment softmax skips the max subtraction: logits for this model are
in [-6, 6], exp() is safe, softmax is shift-invariant.
"""

import os
from contextlib import ExitStack

import ml_dtypes
import numpy as np

import concourse.bacc as bacc
import concourse.mybir as mybir
import concourse.tile as tile
from concourse.bass_utils import run_bass_kernel_spmd

dt = mybir.dt
AF = mybir.ActivationFunctionType
ALU = mybir.AluOpType
BF16 = ml_dtypes.bfloat16

# ---------------- problem constants (hardcoded per contract) ----------------
N = 20000
HID = 128
HEADS = 8
C1 = 64
C2 = 32
D1 = HEADS * C1  # 512
D2 = HEADS * C2  # 256

NCORES = 8
NPC = N // NCORES          # 2500 nodes per core
WNODES = 128               # nodes per window
NW = -(-NPC // WNODES)     # 20 windows per core
NPAD = NW * WNODES         # 2560 padded node slots per core
SBE = 512                  # edges per superblock

LAST_RESULTS = []          # BassKernelResults of the most recent kernel() call


def _bf(x):
    return np.ascontiguousarray(np.asarray(x, np.float32).astype(BF16))


def _f32(x):
    return np.ascontiguousarray(np.asarray(x, np.float32))


def _perm(D, H):
    """Head-interleave permutation: interleaved col j holds original col
    (j%H)*C + j//H  (i.e. j = c*H + h)."""
    j = np.arange(D)
    return (j % H) * (D // H) + j // H


PERM1 = _perm(D1, HEADS)
PERM2 = _perm(D2, HEADS)


def _att_bd(att, D):
    """[H, C] -> [128, nG*8] lhsT tiles of the interleaved block-diag."""
    H, C = att.shape
    nG = D // 128
    bd = np.zeros((D, H), np.float32)
    j = np.arange(D)
    bd[j, j % H] = att[j % H, j // H]
    return bd.reshape(nG, 128, H).transpose(1, 0, 2).reshape(128, nG * 8)


def _plan_edges(edge_index):
    """Bucket self-loop-augmented edges by (core, window); pad to NSB
    superblocks of SBE edges.  Returns (NSB, srcs, goh) where
      srcs[c][w] = int64 src node per padded edge slot (0 for pads)
      goh[c]     = [NW, 128, 2*EPW] bf16  (g01t || g01e one-hots)"""
    src = np.concatenate([edge_index[0].astype(np.int64),
                          np.arange(N, dtype=np.int64)])
    dst = np.concatenate([edge_index[1].astype(np.int64),
                          np.arange(N, dtype=np.int64)])
    core = dst // NPC
    dloc = dst - core * NPC
    win = dloc // WNODES
    din = dloc % WNODES

    order = np.lexsort((win, core))
    src, core, win, din = src[order], core[order], win[order], din[order]

    lists = {}
    nsb = 1
    for c in range(NCORES):
        mc = core == c
        sc, wc, dc = src[mc], win[mc], din[mc]
        for w in range(NW):
            mw = wc == w
            lists[(c, w)] = (sc[mw], dc[mw])
            nsb = max(nsb, -(-int(mw.sum()) // SBE))
    epw = nsb * SBE

    srcs, gohs = [], []
    e = np.arange(epw)
    blk, pin = e // 128, e % 128
    for c in range(NCORES):
        sp_all = np.zeros((NW, epw), np.int64)
        goh = np.zeros((NW, 128, 2 * epw), np.float32)
        for w in range(NW):
            s_, d_ = lists[(c, w)]
            ne = len(s_)
            sp_all[w, :ne] = s_
            # g01t[n, e] = (din[e] == n)
            goh[w, d_, np.arange(ne)] = 1.0
            # g01e[e%128, epw + (e//128)*128 + n] = (din[e] == n)
            goh[w, pin[:ne], epw + blk[:ne] * 128 + d_] = 1.0
        srcs.append(sp_all)
        gohs.append(_bf(goh))
    return nsb, srcs, gohs


def _route_edges(table_bf, srcs, nsb, with_ones):
    """Gather table rows into edge order, per core: em||fm per superblock.

    table_bf: [N, D] bf16 (feature cols already head-interleaved)
    returns list of [NW, 128, NSB*SBSZ] bf16 arrays; per-sb slice is
      em [128, 4, DE]  (with_ones: DE=D+8, last 8 cols per block are 1.0 -
                        they carry the softmax denominator through the U
                        matmul; only legal when DE <= 512)
      || fm [128, D//128, 512]"""
    D = table_bf.shape[1]
    DE = D + 8 if with_ones else D
    nG = D // 128
    sbsz = 4 * DE + nG * SBE
    out = []
    for sp_all in srcs:
        gat = table_bf[sp_all.reshape(-1)].reshape(NW, nsb, SBE, D)
        if with_ones:
            gata = np.empty((NW, nsb, SBE, DE), BF16)
            gata[..., :D] = gat
            gata[..., D:] = np.float32(1.0)
        else:
            gata = gat
        # em[p, b, f] = gata[sb, b*128+p, f]
        em = gata.reshape(NW, nsb, 4, 128, DE).transpose(0, 1, 3, 2, 4)
        em = np.ascontiguousarray(em).reshape(NW, nsb, 128, 4 * DE)
        # fm[p, g, e] = gat[sb, e, g*128+p]
        fm = gat.transpose(0, 1, 3, 2).reshape(NW, nsb, nG, 128, SBE)
        fm = np.ascontiguousarray(fm.transpose(0, 1, 3, 2, 4))
        fm = fm.reshape(NW, nsb, 128, nG * SBE)
        both = np.concatenate([em, fm], axis=3)       # [NW, nsb, 128, sbsz]
        both = np.ascontiguousarray(both.transpose(0, 2, 1, 3))
        out.append(both.reshape(NW, 128, nsb * sbsz))
    return out


# ------------------------------ launch A -----------------------------------

def _build_launch_a():
    nc = bacc.Bacc(None, target_bir_lowering=False)
    x_ownT = nc.dram_tensor("x_ownT", [128, NPAD], dt.bfloat16,
                            kind="ExternalInput")
    Wl1p = nc.dram_tensor("Wl1p", [128, D1], dt.bfloat16, kind="ExternalInput")
    Wr1p = nc.dram_tensor("Wr1p", [128, D1], dt.bfloat16, kind="ExternalInput")
    bxr1p = nc.dram_tensor("bxr1p", [128, D1], dt.float32, kind="ExternalInput")
    Wjk0 = nc.dram_tensor("Wjk0", [128, 128], dt.bfloat16, kind="ExternalInput")

    ao = nc.dram_tensor("ao", [NPAD, 2 * D1 + 128], dt.bfloat16,
                        kind="ExternalOutput")

    with tile.TileContext(nc) as tc, ExitStack() as ctx:
        const = ctx.enter_context(tc.tile_pool(name="const", bufs=1))
        sbuf = ctx.enter_context(tc.tile_pool(name="sbuf", bufs=3))
        pp = ctx.enter_context(tc.tile_pool(name="pp", bufs=4, space="PSUM"))
        pps = ctx.enter_context(tc.tile_pool(name="pps", bufs=2, space="PSUM"))

        def cl(name, hdl, shape, dtype):
            t = const.tile(shape, dtype, tag=name)
            nc.sync.dma_start(t[:], hdl[:])
            return t

        wl = cl("wl", Wl1p, [128, D1], dt.bfloat16)
        wr = cl("wr", Wr1p, [128, D1], dt.bfloat16)
        bx = cl("bx", bxr1p, [128, D1], dt.float32)
        wj = cl("wj", Wjk0, [128, 128], dt.bfloat16)
        xo = const.tile([128, NPAD], dt.bfloat16, tag="xo")
        nc.sync.dma_start(xo[:], x_ownT[:])

        outq = [nc.gpsimd, nc.scalar, nc.sync]
        for w in range(NW):
            lhs = xo[:, w * 128:(w + 1) * 128]
            t1 = sbuf.tile([128, 2 * D1 + 128], dt.bfloat16, tag="t1")
            p1 = pp.tile([128, D1], dt.float32, tag="p1")
            nc.tensor.matmul(p1[:], lhsT=lhs, rhs=wl[:], start=True, stop=True)
            nc.vector.tensor_copy(t1[:, :D1], p1[:])
            p2 = pp.tile([128, D1], dt.float32, tag="p1")
            nc.tensor.matmul(p2[:], lhsT=lhs, rhs=wr[:], start=True, stop=True)
            nc.vector.tensor_tensor(out=t1[:, D1:2 * D1], in0=p2[:], in1=bx[:],
                                    op=ALU.add)
            p3 = pps.tile([128, 128], dt.float32, tag="p3")
            nc.tensor.matmul(p3[:], lhsT=lhs, rhs=wj[:], start=True, stop=True)
            nc.scalar.activation(t1[:, 2 * D1:], p3[:], AF.Copy)
            outq[w % 3].dma_start(ao[w * 128:(w + 1) * 128, :], t1[:])

    nc.compile()
    return nc


# ------------------------- edge-phase launches ------------------------------

def _emit_edge_pipeline(nc, pools, cfg):
    """Software-pipelined edge phase + window epilogues for one GAT layer.

    Pipeline stages (each lags the previous by one superblock iteration):
      phase1(k):  ef DMA, s matmuls (xr scatter + fm accum), Prelu, lg, Exp
      phase2a(k): ept transpose minis, expe copy, pr = em*expe (DVE 2x)
      phase2b(k): U += g01e.T @ pr  (also accumulates the denominator via
                  the ones columns baked into em)
    epi_v(w) is emitted right after phase2b(w, NSB-1); on_h(w) two
    iterations later so the PE never waits on the DVE elu chain."""
    sbuf, empool, gohpool = pools["sbuf"], pools["em"], pools["goh"]
    ppS, ppLG, ppE, ppU = (pools["ppS"], pools["ppLG"], pools["ppE"],
                           pools["ppU"])
    ppDN = pools.get("ppDN")
    D, CH, NSB = cfg["D"], cfg["CH"], cfg["NSB"]
    merged = cfg["merged_dn"]         # denominator rides in U's ones columns
    DE = D + 8 if merged else D
    nG = D // 128
    EPW = NSB * SBE
    SBSZ = 4 * DE + nG * SBE  # per-sb free elements: em (4*DE) || fm (nG*SBE)
    ident = cfg["ident"]

    state = {}

    def phase1(w, sb, ef):
        goh_t = state[("goh", w)]
        lg = ppLG.tile([8, SBE], dt.float32, tag="lg")
        ss, lrs = [], []
        for g in range(nG):
            s = ppS.tile([128, SBE], dt.float32, tag="s")
            nc.tensor.matmul(
                s[:], lhsT=cfg["xr_tile"][:, w * D + g * 128:w * D + (g + 1) * 128],
                rhs=goh_t[:, sb * SBE:(sb + 1) * SBE], start=True, stop=False)
            nc.tensor.matmul(
                s[:], lhsT=ident[:],
                rhs=ef[:, 4 * DE + g * SBE:4 * DE + (g + 1) * SBE],
                start=False, stop=True)
            lr = sbuf.tile([128, SBE], dt.bfloat16, tag="lr")
            nc.scalar.activation(lr[:], s[:], AF.Prelu, alpha=0.2)
            ss.append(s)
            lrs.append(lr)
            # lag the lg matmul one group behind the s matmuls so the PE
            # never waits on the Prelu
            if g >= 1:
                nc.tensor.matmul(lg[:],
                                 lhsT=cfg["att_tile"][:, (g - 1) * 8:g * 8],
                                 rhs=lrs[g - 1][:], start=(g == 1), stop=False)
        nc.tensor.matmul(lg[:], lhsT=cfg["att_tile"][:, (nG - 1) * 8:nG * 8],
                         rhs=lrs[nG - 1][:], start=(nG == 1), stop=True)
        return ef, lg

    def emit_exp(lg):
        expf = sbuf.tile([8, SBE], dt.bfloat16, tag="expf")
        nc.scalar.activation(expf[:], lg[:], AF.Exp)
        return expf

    def phase2a(w, sb, ef, expf):
        ept = ppE.tile([128, 32], dt.float32, tag="ept")
        for b in range(4):
            nc.tensor.matmul(ept[:, b * 8:(b + 1) * 8],
                             lhsT=expf[:, b * 128:(b + 1) * 128],
                             rhs=ident[:8, :8],
                             start=(b == 0), stop=(b == 3))
        expe = sbuf.tile([128, 32], dt.bfloat16, tag="expe")
        nc.scalar.activation(expe[:], ept[:], AF.Copy)
        CHE = CH + 1 if merged else CH
        pr = sbuf.tile([128, 4 * DE], dt.bfloat16, tag="pr")
        for b in range(4):
            nc.vector.tensor_tensor(
                out=pr[:, b * DE:(b + 1) * DE]
                    .rearrange("p (c h) -> p c h", h=8),
                in0=ef[:, b * DE:(b + 1) * DE]
                    .rearrange("p (c h) -> p c h", h=8),
                in1=expe[:, b * 8:(b + 1) * 8].unsqueeze(1)
                    .broadcast_to([128, CHE, 8]),
                op=ALU.mult)
        return pr, expe

    def phase2b(w, sb, pr, expe):
        goh_t = state[("goh", w)]
        U, dn = state[("U", w)]
        for b in range(4):
            lh = goh_t[:, EPW + (sb * 4 + b) * 128:EPW + (sb * 4 + b + 1) * 128]
            first = (sb == 0 and b == 0)
            last = (sb == NSB - 1 and b == 3)
            nc.tensor.matmul(U[:], lhsT=lh, rhs=pr[:, b * DE:(b + 1) * DE],
                             start=first, stop=last)
            if not merged:
                nc.tensor.matmul(dn[:], lhsT=lh,
                                 rhs=expe[:, b * 8:(b + 1) * 8],
                                 start=first, stop=last)

    def epi_v(w):
        U, dn = state[("U", w)]
        dns = sbuf.tile([128, 8], dt.float32, tag="dns")
        nc.vector.tensor_scalar_max(dns[:], U[:, D:DE] if merged else dn[:],
                                    1e-30)
        rd = sbuf.tile([128, 8], dt.float32, tag="rd")
        nc.vector.reciprocal(rd[:], dns[:])
        v = sbuf.tile([128, D], dt.float32, tag="v")
        nc.vector.tensor_tensor(
            out=v[:].rearrange("p (c h) -> p c h", h=8),
            in0=(U[:, :D] if merged else U[:])
                .rearrange("p (c h) -> p c h", h=8),
            in1=rd[:].unsqueeze(1).broadcast_to([128, CH, 8]),
            op=ALU.mult)
        # elu(vb) = max(vb, exp(min(vb, 0)) - 1)
        vb = sbuf.tile([128, D], dt.float32, tag="vb")
        nc.vector.tensor_tensor(out=vb[:], in0=v[:], in1=cfg["biash_tile"][:],
                                op=ALU.add)
        m = sbuf.tile([128, D], dt.float32, tag="m")
        nc.vector.tensor_scalar_min(m[:], vb[:], 0.0)
        em_ = sbuf.tile([128, D], dt.float32, tag="em_")
        nc.scalar.activation(em_[:], m[:], AF.Exp)
        h = sbuf.tile([128, D], dt.bfloat16, tag="h")
        nc.vector.scalar_tensor_tensor(out=h[:], in0=em_[:], scalar=-1.0,
                                       op0=ALU.add, in1=vb[:], op1=ALU.max)
        state[("h", w)] = h

    ef_q = []

    def prefetch(w, sb):
        # issued one full iteration ahead of use so DMA transfers are hidden
        if sb == 0:
            goh_t = gohpool.tile([128, 2 * EPW], dt.bfloat16, tag="goh")
            nc.gpsimd.dma_start(goh_t[:], cfg["goh_dram"][w])
            state[("goh", w)] = goh_t
            if cfg.get("load_xr"):
                cfg["load_xr"](w)
        ef = empool.tile([128, SBSZ], dt.bfloat16, tag="ef")
        nc.gpsimd.dma_start(
            ef[:], cfg["emfm_dram"][w][:, sb * SBSZ:(sb + 1) * SBSZ])
        ef_q.append(ef)

    def begin_window(w):
        U = ppU.tile([128, DE], dt.float32, tag="U")
        dn = None
        if not merged:
            dn = ppDN.tile([128, 8], dt.float32, tag="dn")
        state[("U", w)] = (U, dn)

    items = [(w, sb) for w in range(NW) for sb in range(NSB)]
    fifo_a, fifo_b = [], []   # pending phase2a / phase2b work
    epi_cd = []               # [w, countdown] until on_h emission
    prefetch(*items[0])

    def tick():
        if len(fifo_b) > 1:
            w, sb, pr, expe = fifo_b.pop(0)
            phase2b(w, sb, pr, expe)
            if sb == NSB - 1:
                epi_v(w)
                epi_cd.append([w, 2])
        for e in epi_cd:
            e[1] -= 1
        while epi_cd and epi_cd[0][1] <= 0:
            w = epi_cd.pop(0)[0]
            cfg["on_h"](w, state.pop(("h", w)))
            del state[("goh", w)], state[("U", w)]

    for i, (w, sb) in enumerate(items):
        if sb == 0:
            begin_window(w)
        if i + 1 < len(items):
            prefetch(*items[i + 1])
        ef, lg = phase1(w, sb, ef_q.pop(0))
        if i == 0 and cfg.get("late_consts"):
            cfg["late_consts"]()
        if len(fifo_a) > 0:
            pw, psb, pef, pexpf = fifo_a.pop(0)
            pr, expe = phase2a(pw, psb, pef, pexpf)
            fifo_b.append((pw, psb, pr, expe))
        # exp is emitted after phase2a so the previous superblock's expe
        # copy is not queued behind it on the scalar engine
        fifo_a.append((w, sb, ef, emit_exp(lg)))
        tick()
    # drain
    while fifo_a:
        pw, psb, pef, pexpf = fifo_a.pop(0)
        pr, expe = phase2a(pw, psb, pef, pexpf)
        fifo_b.append((pw, psb, pr, expe))
    while fifo_b:
        w, sb, pr, expe = fifo_b.pop(0)
        phase2b(w, sb, pr, expe)
        if sb == NSB - 1:
            epi_v(w)
            epi_cd.append([w, 0])
    while epi_cd:
        w = epi_cd.pop(0)[0]
        cfg["on_h"](w, state.pop(("h", w)))
        del state[("goh", w)], state[("U", w)]


def _build_launch_b(NSB):
    EPW = NSB * SBE
    nc = bacc.Bacc(None, target_bir_lowering=False)

    emfm = nc.dram_tensor("emfm", [NW, 128, NSB * (4 * D1 + 4 * SBE)],
                          dt.bfloat16, kind="ExternalInput")
    goh = nc.dram_tensor("goh", [NW, 128, 2 * EPW], dt.bfloat16,
                         kind="ExternalInput")
    xr1 = nc.dram_tensor("xr1", [NW, 128, D1], dt.bfloat16,
                         kind="ExternalInput")
    jk0 = nc.dram_tensor("jk0", [NW, 128, 128], dt.bfloat16,
                         kind="ExternalInput")
    att1bd = nc.dram_tensor("att1bd", [128, 32], dt.bfloat16,
                            kind="ExternalInput")
    biash1 = nc.dram_tensor("biash1", [128, D1], dt.float32,
                            kind="ExternalInput")
    identI = nc.dram_tensor("identI", [128, 128], dt.bfloat16,
                            kind="ExternalInput")
    Wl2p = nc.dram_tensor("Wl2p", [128, 4 * D2], dt.bfloat16,
                          kind="ExternalInput")
    Wr2p = nc.dram_tensor("Wr2p", [128, 4 * D2], dt.bfloat16,
                          kind="ExternalInput")
    bxr2p = nc.dram_tensor("bxr2p", [128, D2], dt.float32,
                           kind="ExternalInput")
    Wjk1p = nc.dram_tensor("Wjk1p", [128, 4 * 128], dt.bfloat16,
                           kind="ExternalInput")

    xl2_o = nc.dram_tensor("xl2_o", [NPAD, D2], dt.bfloat16,
                           kind="ExternalOutput")
    xr2_o = nc.dram_tensor("xr2_o", [NPAD, D2], dt.bfloat16,
                           kind="ExternalOutput")
    jk01_o = nc.dram_tensor("jk01_o", [NPAD, 128], dt.float32,
                            kind="ExternalOutput")

    with tile.TileContext(nc) as tc, ExitStack() as ctx:
        const = ctx.enter_context(tc.tile_pool(name="const", bufs=1))
        sbuf = ctx.enter_context(tc.tile_pool(name="sbuf", bufs=3))
        empool = ctx.enter_context(tc.tile_pool(name="em", bufs=3))
        gohpool = ctx.enter_context(tc.tile_pool(name="goh", bufs=2))
        ppS = ctx.enter_context(tc.tile_pool(name="ppS", bufs=2, space="PSUM"))
        ppLG = ctx.enter_context(tc.tile_pool(name="ppLG", bufs=1, space="PSUM"))
        ppE = ctx.enter_context(tc.tile_pool(name="ppE", bufs=1, space="PSUM"))
        ppU = ctx.enter_context(tc.tile_pool(name="ppU", bufs=2, space="PSUM"))
        ppDN = ctx.enter_context(tc.tile_pool(name="ppDN", bufs=2, space="PSUM"))

        def cl(name, hdl, shape, dtype):
            t = const.tile(shape, dtype, tag=name)
            nc.sync.dma_start(t[:], hdl[:])
            return t

        ident = cl("ident", identI, [128, 128], dt.bfloat16)
        att1_t = cl("att1", att1bd, [128, 32], dt.bfloat16)
        bh1_t = const.tile([128, D1], dt.float32, tag="bh1")
        wl2_t = const.tile([128, 4 * D2], dt.bfloat16, tag="wl2")
        wr2_t = const.tile([128, 4 * D2], dt.bfloat16, tag="wr2")
        bxr2_t = const.tile([128, D2], dt.float32, tag="bxr2")
        wjk1_t = const.tile([128, 4 * 128], dt.bfloat16, tag="wjk1")
        xr1_t = const.tile([128, NW * D1], dt.bfloat16, tag="xr1t")

        def late_consts():
            nc.sync.dma_start(bh1_t[:], biash1[:])
            nc.sync.dma_start(wl2_t[:], Wl2p[:])
            nc.sync.dma_start(wr2_t[:], Wr2p[:])
            nc.sync.dma_start(bxr2_t[:], bxr2p[:])
            nc.sync.dma_start(wjk1_t[:], Wjk1p[:])

        def load_xr(w):
            nc.sync.dma_start(xr1_t[:, w * D1:(w + 1) * D1], xr1[w])

        def on_h(w, h):
            # xl2 = h@Wl2p ; xr2 = h@Wr2p + b ; jk01 = jk0 + h@Wjk1p
            p_xl2 = ppS.tile([128, D2], dt.float32, tag="s")
            p_xr2 = ppS.tile([128, D2], dt.float32, tag="s")
            p_jk = ppE.tile([128, 128], dt.float32, tag="ept")
            for g in range(4):
                tp = ppLG.tile([128, 128], dt.float32, tag="lg")
                nc.tensor.matmul(tp[:], lhsT=h[:, g * 128:(g + 1) * 128],
                                 rhs=ident[:], start=True, stop=True)
                hTs = sbuf.tile([128, 128], dt.bfloat16, tag="hT")
                nc.vector.tensor_copy(hTs[:], tp[:])
                nc.tensor.matmul(p_xl2[:], lhsT=hTs[:],
                                 rhs=wl2_t[:, g * D2:(g + 1) * D2],
                                 start=(g == 0), stop=(g == 3))
                nc.tensor.matmul(p_xr2[:], lhsT=hTs[:],
                                 rhs=wr2_t[:, g * D2:(g + 1) * D2],
                                 start=(g == 0), stop=(g == 3))
                nc.tensor.matmul(p_jk[:], lhsT=hTs[:],
                                 rhs=wjk1_t[:, g * 128:(g + 1) * 128],
                                 start=(g == 0), stop=(g == 3))
            o_xl2 = sbuf.tile([128, D2], dt.bfloat16, tag="oxl2")
            nc.any.tensor_copy(o_xl2[:], p_xl2[:])
            nc.gpsimd.dma_start(xl2_o[w * 128:(w + 1) * 128, :], o_xl2[:])
            o_xr2 = sbuf.tile([128, D2], dt.bfloat16, tag="oxr2")
            nc.vector.tensor_tensor(out=o_xr2[:], in0=p_xr2[:], in1=bxr2_t[:],
                                    op=ALU.add)
            nc.gpsimd.dma_start(xr2_o[w * 128:(w + 1) * 128, :], o_xr2[:])
            jk0_t = sbuf.tile([128, 128], dt.bfloat16, tag="jk0")
            nc.gpsimd.dma_start(jk0_t[:], jk0[w])
            o_jk = sbuf.tile([128, 128], dt.float32, tag="ojk")
            nc.vector.tensor_tensor(out=o_jk[:], in0=p_jk[:], in1=jk0_t[:],
                                    op=ALU.add)
            nc.gpsimd.dma_start(jk01_o[w * 128:(w + 1) * 128, :], o_jk[:])

        pools = dict(sbuf=sbuf, em=empool, goh=gohpool, ppS=ppS, ppLG=ppLG,
                     ppE=ppE, ppU=ppU, ppDN=ppDN)
        _emit_edge_pipeline(nc, pools, dict(
            D=D1, CH=C1, NSB=NSB, merged_dn=False,
            emfm_dram=emfm, goh_dram=goh,
            xr_tile=xr1_t, att_tile=att1_t, biash_tile=bh1_t,
            ident=ident, on_h=on_h, late_consts=late_consts, load_xr=load_xr))

    nc.compile()
    return nc


def _build_launch_c(NSB):
    EPW = NSB * SBE
    nc = bacc.Bacc(None, target_bir_lowering=False)

    emfm = nc.dram_tensor("emfm", [NW, 128, NSB * (4 * (D2 + 8) + 2 * SBE)],
                          dt.bfloat16, kind="ExternalInput")
    goh = nc.dram_tensor("goh", [NW, 128, 2 * EPW], dt.bfloat16,
                         kind="ExternalInput")
    xr2 = nc.dram_tensor("xr2", [NW, 128, D2], dt.bfloat16,
                         kind="ExternalInput")
    jk01 = nc.dram_tensor("jk01", [NW, 128, 128], dt.float32,
                          kind="ExternalInput")
    att2bd = nc.dram_tensor("att2bd", [128, 16], dt.bfloat16,
                            kind="ExternalInput")
    biash2 = nc.dram_tensor("biash2", [128, D2], dt.float32,
                            kind="ExternalInput")
    identI = nc.dram_tensor("identI", [128, 128], dt.bfloat16,
                            kind="ExternalInput")
    Wjk2p = nc.dram_tensor("Wjk2p", [128, 2 * 128], dt.bfloat16,
                           kind="ExternalInput")
    bjk_r = nc.dram_tensor("bjk_r", [1, 128], dt.bfloat16,
                           kind="ExternalInput")
    ones1d = nc.dram_tensor("ones1", [1, 128], dt.bfloat16,
                            kind="ExternalInput")

    out_o = nc.dram_tensor("out_o", [NPAD, 128], dt.float32,
                           kind="ExternalOutput")

    with tile.TileContext(nc) as tc, ExitStack() as ctx:
        const = ctx.enter_context(tc.tile_pool(name="const", bufs=1))
        sbuf = ctx.enter_context(tc.tile_pool(name="sbuf", bufs=3))
        empool = ctx.enter_context(tc.tile_pool(name="em", bufs=3))
        gohpool = ctx.enter_context(tc.tile_pool(name="goh", bufs=2))
        ppS = ctx.enter_context(tc.tile_pool(name="ppS", bufs=2, space="PSUM"))
        ppLG = ctx.enter_context(tc.tile_pool(name="ppLG", bufs=1, space="PSUM"))
        ppE = ctx.enter_context(tc.tile_pool(name="ppE", bufs=1, space="PSUM"))
        ppU = ctx.enter_context(tc.tile_pool(name="ppU", bufs=2, space="PSUM"))

        def cl(name, hdl, shape, dtype):
            t = const.tile(shape, dtype, tag=name)
            nc.sync.dma_start(t[:], hdl[:])
            return t

        ident = cl("ident", identI, [128, 128], dt.bfloat16)
        att2_t = cl("att2", att2bd, [128, 16], dt.bfloat16)
        bh2_t = const.tile([128, D2], dt.float32, tag="bh2")
        wjk2_t = const.tile([128, 2 * 128], dt.bfloat16, tag="wjk2")
        bjkr_t = cl("bjkr", bjk_r, [1, 128], dt.bfloat16)
        ones1 = cl("ones1", ones1d, [1, 128], dt.bfloat16)
        xr2_t = const.tile([128, NW * D2], dt.bfloat16, tag="xr2t")

        def late_consts():
            nc.sync.dma_start(bh2_t[:], biash2[:])
            nc.sync.dma_start(wjk2_t[:], Wjk2p[:])

        def load_xr(w):
            nc.sync.dma_start(xr2_t[:, w * D2:(w + 1) * D2], xr2[w])

        def on_h(w, h):
            p_out = ppS.tile([128, 128], dt.float32, tag="s")
            nc.tensor.matmul(p_out[:], lhsT=ones1[:], rhs=bjkr_t[:],
                             start=True, stop=False)
            for g in range(2):
                tp = ppLG.tile([128, 128], dt.float32, tag="lg")
                nc.tensor.matmul(tp[:], lhsT=h[:, g * 128:(g + 1) * 128],
                                 rhs=ident[:], start=True, stop=True)
                hTs = sbuf.tile([128, 128], dt.bfloat16, tag="hT")
                nc.vector.tensor_copy(hTs[:], tp[:])
                nc.tensor.matmul(p_out[:], lhsT=hTs[:],
                                 rhs=wjk2_t[:, g * 128:(g + 1) * 128],
                                 start=False, stop=(g == 1))
            jk_t = sbuf.tile([128, 128], dt.float32, tag="jkt")
            nc.gpsimd.dma_start(jk_t[:], jk01[w])
            o_t = sbuf.tile([128, 128], dt.float32, tag="ot")
            nc.vector.tensor_tensor(out=o_t[:], in0=p_out[:], in1=jk_t[:],
                                    op=ALU.add)
            nc.gpsimd.dma_start(out_o[w * 128:(w + 1) * 128, :], o_t[:])

        pools = dict(sbuf=sbuf, em=empool, goh=gohpool, ppS=ppS, ppLG=ppLG,
                     ppE=ppE, ppU=ppU)
        _emit_edge_pipeline(nc, pools, dict(
            D=D2, CH=C2, NSB=NSB, merged_dn=True,
            emfm_dram=emfm, goh_dram=goh,
            xr_tile=xr2_t, att_tile=att2_t, biash_tile=bh2_t,
            ident=ident, on_h=on_h, late_consts=late_consts, load_xr=load_xr))

    nc.compile()
    return nc


_PROGRAM_CACHE = {}


def kernel(x, edge_index, Wl1, bl1, Wr1, br1, att1, bias1,
           Wl2, bl2, Wr2, br2, att2, bias2, Wjk, bjk):
    global LAST_RESULTS
    LAST_RESULTS = []
    trace = bool(os.environ.get("GAT_TRACE"))

    x = _f32(x)
    Wl1, Wr1 = _f32(Wl1), _f32(Wr1)
    Wl2, Wr2 = _f32(Wl2), _f32(Wr2)
    Wjk = _f32(Wjk)
    NSB, srcs, gohs = _plan_edges(np.asarray(edge_index))

    if "A" not in _PROGRAM_CACHE:
        _PROGRAM_CACHE["A"] = _build_launch_a()
    if ("B", NSB) not in _PROGRAM_CACHE:
        _PROGRAM_CACHE[("B", NSB)] = _build_launch_b(NSB)
    if ("C", NSB) not in _PROGRAM_CACHE:
        _PROGRAM_CACHE[("C", NSB)] = _build_launch_c(NSB)

    ident = np.eye(128, dtype=np.float32)

    # ---------------- launch A: per-node transforms ----------------
    common_a = dict(
        Wl1p=_bf(Wl1[:, PERM1]),
        Wr1p=_bf(Wr1[:, PERM1]),
        bxr1p=_f32(np.tile((np.asarray(bl1) + np.asarray(br1))[PERM1][None, :],
                           (128, 1))),
        Wjk0=_bf(Wjk[:128]),
    )
    in_maps_a = []
    for c in range(NCORES):
        xo = np.zeros((128, NPAD), np.float32)
        xo[:, :NPC] = x[c * NPC:(c + 1) * NPC].T
        in_maps_a.append(dict(common_a, x_ownT=_bf(xo)))

    res_a = run_bass_kernel_spmd(_PROGRAM_CACHE["A"], in_maps_a,
                                 core_ids=list(range(NCORES)), trace=trace)
    LAST_RESULTS.append(res_a)

    # ---------------- host routing for layer 1 ----------------
    ao = [np.asarray(res_a.results[c]["ao"]) for c in range(NCORES)]
    xl1_all = np.concatenate([a[:NPC, :D1] for a in ao], axis=0)
    emfm1 = _route_edges(xl1_all, srcs, NSB, with_ones=False)

    common_b = dict(
        att1bd=_bf(_att_bd(np.asarray(att1), D1)),
        biash1=_f32(np.tile((np.asarray(bl1) + np.asarray(bias1))[PERM1][None, :],
                            (128, 1))),
        identI=_bf(ident),
        Wl2p=_bf(Wl2[PERM1][:, PERM2].reshape(4, 128, D2)
                 .transpose(1, 0, 2).reshape(128, 4 * D2)),
        Wr2p=_bf(Wr2[PERM1][:, PERM2].reshape(4, 128, D2)
                 .transpose(1, 0, 2).reshape(128, 4 * D2)),
        bxr2p=_f32(np.tile((np.asarray(bl2) + np.asarray(br2))[PERM2][None, :],
                           (128, 1))),
        Wjk1p=_bf(Wjk[128:128 + D1][PERM1].reshape(4, 128, 128)
                  .transpose(1, 0, 2).reshape(128, 4 * 128)),
    )
    in_maps_b = []
    for c in range(NCORES):
        in_maps_b.append(dict(
            common_b,
            emfm=emfm1[c],
            goh=gohs[c],
            xr1=np.ascontiguousarray(ao[c][:, D1:2 * D1].reshape(NW, 128, D1)),
            jk0=np.ascontiguousarray(ao[c][:, 2 * D1:].reshape(NW, 128, 128)),
        ))

    res_b = run_bass_kernel_spmd(_PROGRAM_CACHE[("B", NSB)], in_maps_b,
                                 core_ids=list(range(NCORES)), trace=trace)
    LAST_RESULTS.append(res_b)

    # ---------------- host routing for layer 2 ----------------
    xl2_all = np.concatenate(
        [np.asarray(res_b.results[c]["xl2_o"])[:NPC] for c in range(NCORES)],
        axis=0)                                   # [N, 256] bf16, interleaved
    emfm2 = _route_edges(xl2_all, srcs, NSB, with_ones=True)

    common_c = dict(
        att2bd=_bf(_att_bd(np.asarray(att2), D2)),
        biash2=_f32(np.tile((np.asarray(bl2) + np.asarray(bias2))[PERM2][None, :],
                            (128, 1))),
        identI=_bf(ident),
        Wjk2p=_bf(Wjk[128 + D1:][PERM2].reshape(2, 128, 128)
                  .transpose(1, 0, 2).reshape(128, 2 * 128)),
        bjk_r=_bf(np.asarray(bjk)[None, :]),
        ones1=_bf(np.ones((1, 128), np.float32)),
    )
    in_maps_c = []
    for c in range(NCORES):
        in_maps_c.append(dict(
            common_c,
            emfm=emfm2[c],
            goh=gohs[c],
            xr2=np.asarray(res_b.results[c]["xr2_o"]).reshape(NW, 128, D2),
            jk01=_f32(np.asarray(res_b.results[c]["jk01_o"])
                      .reshape(NW, 128, 128)),
        ))

    res_c = run_bass_kernel_spmd(_PROGRAM_CACHE[("C", NSB)], in_maps_c,
                                 core_ids=list(range(NCORES)), trace=trace)
    LAST_RESULTS.append(res_c)

    out = np.concatenate(
        [np.asarray(res_c.results[c]["out_o"])[:NPC] for c in range(NCORES)],
        axis=0)
    return np.ascontiguousarray(out, dtype=np.float32)


# revision 44
# speedup vs baseline: 1.1548x; 1.0183x over previous
"""Trainium2 Bass kernel for a 2-layer GATv2 + JumpingKnowledge GNN.

Strategy (8 NeuronCores, dst-node sharding, 3 launches, zero on-device
gathers):
  - Host: add self loops, bucket edges by (core, 128-node dst window), pad
    every window to NSB 512-edge superblocks.  Build per-window one-hot
    matrices (node-major g01t and edge-major g01e) on the host.
  - Launch A (node-sharded): xl1 = x@Wl1, xr1 = x@Wr1 + bl1+br1,
    jk0 = x@Wjk0 for owned nodes.  Pure per-node GEMMs, ~40us.
  - Host: route xl1 rows into edge order (halo exchange): ship BOTH an
    edge-major copy (for the alpha-weighted message aggregation) and a
    feature-major copy (for the attention-logit pipeline).  Pure
    permutation of device-computed data - no FLOPs on host.
  - Launch B: layer-1 edge phase + h1 + layer-2 node transforms
    (xl2/xr2/jk01).
  - Host: route xl2 rows into edge order (same shapes as layer 1 / 2).
  - Launch C: layer-2 edge phase + JumpingKnowledge output projection.

Edge phase per 512-edge superblock (no gathers, no PE transposes):
  s_fm[g]   = xr_win[:,g] @ g01t  +  I @ xl_fm[g]        (PE, 2 matmuls/group)
  lr        = Prelu(s_fm, 0.2)                           (ACT)
  lg       += att_bd[g].T @ lr                           (PE)
  expf      = Exp(lg)                                    (ACT)
  expe      = transpose(expf) via 4 tiny PE matmuls      (PE)
  pr[b]     = xl_em[b] * expe[b]  (head-broadcast)       (DVE, 2x mode)
  U        += g01e[b].T @ pr[b] ; dn += g01e[b].T @ expe (PE, window accum)
Window epilogue: h = elu(U/dn + bias), then next-layer node GEMMs.

All feature axes use a head-interleaved order f=(c*H+h) so the DVE
broadcast multiply has innermost stride 1 (2x DVE perf mode).  Every
weight matrix is permuted accordingly on the host; the final output is
un-permuted (Wjk rows permuted to compensate).

The segment softmax skips the max subtraction: logits for this model are
in [-6, 6], exp() is safe, softmax is shift-invariant.
"""

import os
from contextlib import ExitStack

import ml_dtypes
import numpy as np

import concourse.bacc as bacc
import concourse.mybir as mybir
import concourse.tile as tile
from concourse.bass_utils import run_bass_kernel_spmd

dt = mybir.dt
AF = mybir.ActivationFunctionType
ALU = mybir.AluOpType
BF16 = ml_dtypes.bfloat16

# ---------------- problem constants (hardcoded per contract) ----------------
N = 20000
HID = 128
HEADS = 8
C1 = 64
C2 = 32
D1 = HEADS * C1  # 512
D2 = HEADS * C2  # 256

NCORES = 8
NPC = N // NCORES          # 2500 nodes per core
WNODES = 128               # nodes per window
NW = -(-NPC // WNODES)     # 20 windows per core
NPAD = NW * WNODES         # 2560 padded node slots per core
SBE = 512                  # edges per superblock

LAST_RESULTS = []          # BassKernelResults of the most recent kernel() call


def _bf(x):
    return np.ascontiguousarray(np.asarray(x, np.float32).astype(BF16))


def _f32(x):
    return np.ascontiguousarray(np.asarray(x, np.float32))


def _perm(D, H):
    """Head-interleave permutation: interleaved col j holds original col
    (j%H)*C + j//H  (i.e. j = c*H + h)."""
    j = np.arange(D)
    return (j % H) * (D // H) + j // H


PERM1 = _perm(D1, HEADS)
PERM2 = _perm(D2, HEADS)


def _att_bd(att, D):
    """[H, C] -> [128, nG*8] lhsT tiles of the interleaved block-diag."""
    H, C = att.shape
    nG = D // 128
    bd = np.zeros((D, H), np.float32)
    j = np.arange(D)
    bd[j, j % H] = att[j % H, j // H]
    return bd.reshape(nG, 128, H).transpose(1, 0, 2).reshape(128, nG * 8)


def _plan_edges(edge_index):
    """Bucket self-loop-augmented edges by (core, window); pad to NSB
    superblocks of SBE edges.  Returns (NSB, srcs, goh) where
      srcs[c][w] = int64 src node per padded edge slot (0 for pads)
      goh[c]     = [NW, 128, 2*EPW] bf16  (g01t || g01e one-hots)"""
    src = np.concatenate([edge_index[0].astype(np.int64),
                          np.arange(N, dtype=np.int64)])
    dst = np.concatenate([edge_index[1].astype(np.int64),
                          np.arange(N, dtype=np.int64)])
    core = dst // NPC
    dloc = dst - core * NPC
    win = dloc // WNODES
    din = dloc % WNODES

    order = np.lexsort((win, core))
    src, core, win, din = src[order], core[order], win[order], din[order]

    lists = {}
    nsb = 1
    for c in range(NCORES):
        mc = core == c
        sc, wc, dc = src[mc], win[mc], din[mc]
        for w in range(NW):
            mw = wc == w
            lists[(c, w)] = (sc[mw], dc[mw])
            nsb = max(nsb, -(-int(mw.sum()) // SBE))
    epw = nsb * SBE

    srcs, gohs = [], []
    e = np.arange(epw)
    blk, pin = e // 128, e % 128
    for c in range(NCORES):
        sp_all = np.zeros((NW, epw), np.int64)
        goh = np.zeros((NW, 128, 2 * epw), np.float32)
        for w in range(NW):
            s_, d_ = lists[(c, w)]
            ne = len(s_)
            sp_all[w, :ne] = s_
            # g01t[n, e] = (din[e] == n)
            goh[w, d_, np.arange(ne)] = 1.0
            # g01e[e%128, epw + (e//128)*128 + n] = (din[e] == n)
            goh[w, pin[:ne], epw + blk[:ne] * 128 + d_] = 1.0
        srcs.append(sp_all)
        gohs.append(_bf(goh))
    return nsb, srcs, gohs


def _route_edges(table_bf, srcs, nsb, with_ones):
    """Gather table rows into edge order, per core: em||fm per superblock.

    table_bf: [N, D] bf16 (feature cols already head-interleaved)
    returns list of [NW, 128, NSB*SBSZ] bf16 arrays; per-sb slice is
      em [128, 4, DE]  (with_ones: DE=D+8, last 8 cols per block are 1.0 -
                        they carry the softmax denominator through the U
                        matmul; only legal when DE <= 512)
      || fm [128, D//128, 512]"""
    D = table_bf.shape[1]
    DE = D + 8 if with_ones else D
    nG = D // 128
    sbsz = 4 * DE + nG * SBE
    out = []
    for sp_all in srcs:
        gat = table_bf[sp_all.reshape(-1)].reshape(NW, nsb, SBE, D)
        if with_ones:
            gata = np.empty((NW, nsb, SBE, DE), BF16)
            gata[..., :D] = gat
            gata[..., D:] = np.float32(1.0)
        else:
            gata = gat
        # em[p, b, f] = gata[sb, b*128+p, f]
        em = gata.reshape(NW, nsb, 4, 128, DE).transpose(0, 1, 3, 2, 4)
        em = np.ascontiguousarray(em).reshape(NW, nsb, 128, 4 * DE)
        # fm[p, g, e] = gat[sb, e, g*128+p]
        fm = gat.transpose(0, 1, 3, 2).reshape(NW, nsb, nG, 128, SBE)
        fm = np.ascontiguousarray(fm.transpose(0, 1, 3, 2, 4))
        fm = fm.reshape(NW, nsb, 128, nG * SBE)
        both = np.concatenate([em, fm], axis=3)       # [NW, nsb, 128, sbsz]
        both = np.ascontiguousarray(both.transpose(0, 2, 1, 3))
        out.append(both.reshape(NW, 128, nsb * sbsz))
    return out


# ------------------------------ launch A -----------------------------------

def _build_launch_a():
    nc = bacc.Bacc(None, target_bir_lowering=False)
    x_ownT = nc.dram_tensor("x_ownT", [128, NPAD], dt.bfloat16,
                            kind="ExternalInput")
    Wl1p = nc.dram_tensor("Wl1p", [128, D1], dt.bfloat16, kind="ExternalInput")
    Wr1p = nc.dram_tensor("Wr1p", [128, D1], dt.bfloat16, kind="ExternalInput")
    bxr1p = nc.dram_tensor("bxr1p", [128, D1], dt.float32, kind="ExternalInput")
    Wjk0 = nc.dram_tensor("Wjk0", [128, 128], dt.bfloat16, kind="ExternalInput")

    ao = nc.dram_tensor("ao", [NPAD, 2 * D1 + 128], dt.bfloat16,
                        kind="ExternalOutput")

    with tile.TileContext(nc) as tc, ExitStack() as ctx:
        const = ctx.enter_context(tc.tile_pool(name="const", bufs=1))
        sbuf = ctx.enter_context(tc.tile_pool(name="sbuf", bufs=3))
        pp = ctx.enter_context(tc.tile_pool(name="pp", bufs=4, space="PSUM"))
        pps = ctx.enter_context(tc.tile_pool(name="pps", bufs=2, space="PSUM"))

        def cl(name, hdl, shape, dtype):
            t = const.tile(shape, dtype, tag=name)
            nc.sync.dma_start(t[:], hdl[:])
            return t

        wl = cl("wl", Wl1p, [128, D1], dt.bfloat16)
        wr = cl("wr", Wr1p, [128, D1], dt.bfloat16)
        bx = cl("bx", bxr1p, [128, D1], dt.float32)
        wj = cl("wj", Wjk0, [128, 128], dt.bfloat16)
        xo = const.tile([128, NPAD], dt.bfloat16, tag="xo")
        nc.sync.dma_start(xo[:], x_ownT[:])

        outq = [nc.gpsimd, nc.scalar, nc.sync]
        for w in range(NW):
            lhs = xo[:, w * 128:(w + 1) * 128]
            t1 = sbuf.tile([128, 2 * D1 + 128], dt.bfloat16, tag="t1")
            p1 = pp.tile([128, D1], dt.float32, tag="p1")
            nc.tensor.matmul(p1[:], lhsT=lhs, rhs=wl[:], start=True, stop=True)
            nc.vector.tensor_copy(t1[:, :D1], p1[:])
            p2 = pp.tile([128, D1], dt.float32, tag="p1")
            nc.tensor.matmul(p2[:], lhsT=lhs, rhs=wr[:], start=True, stop=True)
            nc.vector.tensor_tensor(out=t1[:, D1:2 * D1], in0=p2[:], in1=bx[:],
                                    op=ALU.add)
            p3 = pps.tile([128, 128], dt.float32, tag="p3")
            nc.tensor.matmul(p3[:], lhsT=lhs, rhs=wj[:], start=True, stop=True)
            nc.scalar.activation(t1[:, 2 * D1:], p3[:], AF.Copy)
            outq[w % 3].dma_start(ao[w * 128:(w + 1) * 128, :], t1[:])

    nc.compile()
    return nc


# ------------------------- edge-phase launches ------------------------------

def _emit_edge_pipeline(nc, pools, cfg):
    """Software-pipelined edge phase + window epilogues for one GAT layer.

    Pipeline stages (each lags the previous by one superblock iteration):
      phase1(k):  ef DMA, s matmuls (xr scatter + fm accum), Prelu, lg, Exp
      phase2a(k): ept transpose minis, expe copy, pr = em*expe (DVE 2x)
      phase2b(k): U += g01e.T @ pr  (also accumulates the denominator via
                  the ones columns baked into em)
    epi_v(w) is emitted right after phase2b(w, NSB-1); on_h(w) two
    iterations later so the PE never waits on the DVE elu chain."""
    sbuf, empool, gohpool = pools["sbuf"], pools["em"], pools["goh"]
    ppS, ppLG, ppE, ppU = (pools["ppS"], pools["ppLG"], pools["ppE"],
                           pools["ppU"])
    ppDN = pools.get("ppDN")
    D, CH, NSB = cfg["D"], cfg["CH"], cfg["NSB"]
    merged = cfg["merged_dn"]         # denominator rides in U's ones columns
    DE = D + 8 if merged else D
    nG = D // 128
    EPW = NSB * SBE
    SBSZ = 4 * DE + nG * SBE  # per-sb free elements: em (4*DE) || fm (nG*SBE)
    ident = cfg["ident"]

    state = {}

    def phase1(w, sb, ef):
        goh_t = state[("goh", w)]
        lg = ppLG.tile([8, SBE], dt.float32, tag="lg")
        ss, lrs = [], []
        for g in range(nG):
            s = ppS.tile([128, SBE], dt.float32, tag="s")
            nc.tensor.matmul(
                s[:], lhsT=cfg["xr_tile"][:, w * D + g * 128:w * D + (g + 1) * 128],
                rhs=goh_t[:, sb * SBE:(sb + 1) * SBE], start=True, stop=False)
            nc.tensor.matmul(
                s[:], lhsT=ident[:],
                rhs=ef[:, 4 * DE + g * SBE:4 * DE + (g + 1) * SBE],
                start=False, stop=True)
            lr = sbuf.tile([128, SBE], dt.bfloat16, tag="lr")
            nc.scalar.activation(lr[:], s[:], AF.Prelu, alpha=0.2)
            ss.append(s)
            lrs.append(lr)
            # lag the lg matmul one group behind the s matmuls so the PE
            # never waits on the Prelu
            if g >= 1:
                nc.tensor.matmul(lg[:],
                                 lhsT=cfg["att_tile"][:, (g - 1) * 8:g * 8],
                                 rhs=lrs[g - 1][:], start=(g == 1), stop=False)
        nc.tensor.matmul(lg[:], lhsT=cfg["att_tile"][:, (nG - 1) * 8:nG * 8],
                         rhs=lrs[nG - 1][:], start=(nG == 1), stop=True)
        return ef, lg

    def emit_exp(lg):
        expf = sbuf.tile([8, SBE], dt.bfloat16, tag="expf")
        nc.scalar.activation(expf[:], lg[:], AF.Exp)
        return expf

    def phase2a(w, sb, ef, expf):
        ept = ppE.tile([128, 32], dt.float32, tag="ept")
        for b in range(4):
            nc.tensor.matmul(ept[:, b * 8:(b + 1) * 8],
                             lhsT=expf[:, b * 128:(b + 1) * 128],
                             rhs=ident[:8, :8],
                             start=(b == 0), stop=(b == 3))
        expe = sbuf.tile([128, 32], dt.bfloat16, tag="expe")
        if cfg.get("expe_on_act"):
            nc.scalar.activation(expe[:], ept[:], AF.Copy)
        else:
            nc.vector.tensor_copy(expe[:], ept[:])
        CHE = CH + 1 if merged else CH
        pr = sbuf.tile([128, 4 * DE], dt.bfloat16, tag="pr")
        for b in range(4):
            nc.vector.tensor_tensor(
                out=pr[:, b * DE:(b + 1) * DE]
                    .rearrange("p (c h) -> p c h", h=8),
                in0=ef[:, b * DE:(b + 1) * DE]
                    .rearrange("p (c h) -> p c h", h=8),
                in1=expe[:, b * 8:(b + 1) * 8].unsqueeze(1)
                    .broadcast_to([128, CHE, 8]),
                op=ALU.mult)
        return pr, expe

    def phase2b(w, sb, pr, expe):
        goh_t = state[("goh", w)]
        U, dn = state[("U", w)]
        for b in range(4):
            lh = goh_t[:, EPW + (sb * 4 + b) * 128:EPW + (sb * 4 + b + 1) * 128]
            first = (sb == 0 and b == 0)
            last = (sb == NSB - 1 and b == 3)
            nc.tensor.matmul(U[:], lhsT=lh, rhs=pr[:, b * DE:(b + 1) * DE],
                             start=first, stop=last)
            if not merged:
                nc.tensor.matmul(dn[:], lhsT=lh,
                                 rhs=expe[:, b * 8:(b + 1) * 8],
                                 start=first, stop=last)

    def epi_v(w):
        U, dn = state[("U", w)]
        dns = sbuf.tile([128, 8], dt.float32, tag="dns")
        nc.vector.tensor_scalar_max(dns[:], U[:, D:DE] if merged else dn[:],
                                    1e-30)
        rd = sbuf.tile([128, 8], dt.float32, tag="rd")
        nc.vector.reciprocal(rd[:], dns[:])
        v = sbuf.tile([128, D], dt.float32, tag="v")
        nc.vector.tensor_tensor(
            out=v[:].rearrange("p (c h) -> p c h", h=8),
            in0=(U[:, :D] if merged else U[:])
                .rearrange("p (c h) -> p c h", h=8),
            in1=rd[:].unsqueeze(1).broadcast_to([128, CH, 8]),
            op=ALU.mult)
        # elu(vb) = max(vb, exp(min(vb, 0)) - 1)
        vb = sbuf.tile([128, D], dt.float32, tag="vb")
        nc.vector.tensor_tensor(out=vb[:], in0=v[:], in1=cfg["biash_tile"][:],
                                op=ALU.add)
        m = sbuf.tile([128, D], dt.float32, tag="m")
        nc.vector.tensor_scalar_min(m[:], vb[:], 0.0)
        em_ = sbuf.tile([128, D], dt.float32, tag="em_")
        nc.scalar.activation(em_[:], m[:], AF.Exp)
        h = sbuf.tile([128, D], dt.bfloat16, tag="h")
        nc.vector.scalar_tensor_tensor(out=h[:], in0=em_[:], scalar=-1.0,
                                       op0=ALU.add, in1=vb[:], op1=ALU.max)
        state[("h", w)] = h

    ef_q = []

    def prefetch(w, sb):
        # issued one full iteration ahead of use so DMA transfers are hidden
        if sb == 0:
            goh_t = gohpool.tile([128, 2 * EPW], dt.bfloat16, tag="goh")
            nc.gpsimd.dma_start(goh_t[:], cfg["goh_dram"][w])
            state[("goh", w)] = goh_t
            if cfg.get("load_xr"):
                cfg["load_xr"](w)
        ef = empool.tile([128, SBSZ], dt.bfloat16, tag="ef")
        nc.gpsimd.dma_start(
            ef[:], cfg["emfm_dram"][w][:, sb * SBSZ:(sb + 1) * SBSZ])
        ef_q.append(ef)

    def begin_window(w):
        U = ppU.tile([128, DE], dt.float32, tag="U")
        dn = None
        if not merged:
            dn = ppDN.tile([128, 8], dt.float32, tag="dn")
        state[("U", w)] = (U, dn)

    items = [(w, sb) for w in range(NW) for sb in range(NSB)]
    fifo_a, fifo_b = [], []   # pending phase2a / phase2b work
    epi_cd = []               # [w, countdown] until on_h emission
    prefetch(*items[0])

    def tick():
        if len(fifo_b) > 1:
            w, sb, pr, expe = fifo_b.pop(0)
            phase2b(w, sb, pr, expe)
            if sb == NSB - 1:
                epi_v(w)
                epi_cd.append([w, 2])
        for e in epi_cd:
            e[1] -= 1
        while epi_cd and epi_cd[0][1] <= 0:
            w = epi_cd.pop(0)[0]
            cfg["on_h"](w, state.pop(("h", w)))
            del state[("goh", w)], state[("U", w)]

    for i, (w, sb) in enumerate(items):
        if sb == 0:
            begin_window(w)
        if i + 1 < len(items):
            prefetch(*items[i + 1])
        ef, lg = phase1(w, sb, ef_q.pop(0))
        if i == 0 and cfg.get("late_consts"):
            cfg["late_consts"]()
        if len(fifo_a) > 0:
            pw, psb, pef, pexpf = fifo_a.pop(0)
            pr, expe = phase2a(pw, psb, pef, pexpf)
            fifo_b.append((pw, psb, pr, expe))
        # exp is emitted after phase2a so the previous superblock's expe
        # copy is not queued behind it on the scalar engine
        fifo_a.append((w, sb, ef, emit_exp(lg)))
        tick()
    # drain
    while fifo_a:
        pw, psb, pef, pexpf = fifo_a.pop(0)
        pr, expe = phase2a(pw, psb, pef, pexpf)
        fifo_b.append((pw, psb, pr, expe))
    while fifo_b:
        w, sb, pr, expe = fifo_b.pop(0)
        phase2b(w, sb, pr, expe)
        if sb == NSB - 1:
            epi_v(w)
            epi_cd.append([w, 0])
    while epi_cd:
        w = epi_cd.pop(0)[0]
        cfg["on_h"](w, state.pop(("h", w)))
        del state[("goh", w)], state[("U", w)]


def _build_launch_b(NSB):
    EPW = NSB * SBE
    nc = bacc.Bacc(None, target_bir_lowering=False)

    emfm = nc.dram_tensor("emfm", [NW, 128, NSB * (4 * D1 + 4 * SBE)],
                          dt.bfloat16, kind="ExternalInput")
    goh = nc.dram_tensor("goh", [NW, 128, 2 * EPW], dt.bfloat16,
                         kind="ExternalInput")
    xr1 = nc.dram_tensor("xr1", [NW, 128, D1], dt.bfloat16,
                         kind="ExternalInput")
    jk0 = nc.dram_tensor("jk0", [NW, 128, 128], dt.bfloat16,
                         kind="ExternalInput")
    att1bd = nc.dram_tensor("att1bd", [128, 32], dt.bfloat16,
                            kind="ExternalInput")
    biash1 = nc.dram_tensor("biash1", [128, D1], dt.float32,
                            kind="ExternalInput")
    identI = nc.dram_tensor("identI", [128, 128], dt.bfloat16,
                            kind="ExternalInput")
    Wl2p = nc.dram_tensor("Wl2p", [128, 4 * D2], dt.bfloat16,
                          kind="ExternalInput")
    Wr2p = nc.dram_tensor("Wr2p", [128, 4 * D2], dt.bfloat16,
                          kind="ExternalInput")
    bxr2p = nc.dram_tensor("bxr2p", [128, D2], dt.float32,
                           kind="ExternalInput")
    Wjk1p = nc.dram_tensor("Wjk1p", [128, 4 * 128], dt.bfloat16,
                           kind="ExternalInput")

    xl2_o = nc.dram_tensor("xl2_o", [NPAD, D2], dt.bfloat16,
                           kind="ExternalOutput")
    xr2_o = nc.dram_tensor("xr2_o", [NPAD, D2], dt.bfloat16,
                           kind="ExternalOutput")
    jk01_o = nc.dram_tensor("jk01_o", [NPAD, 128], dt.float32,
                            kind="ExternalOutput")

    with tile.TileContext(nc) as tc, ExitStack() as ctx:
        const = ctx.enter_context(tc.tile_pool(name="const", bufs=1))
        sbuf = ctx.enter_context(tc.tile_pool(name="sbuf", bufs=3))
        empool = ctx.enter_context(tc.tile_pool(name="em", bufs=3))
        gohpool = ctx.enter_context(tc.tile_pool(name="goh", bufs=2))
        ppS = ctx.enter_context(tc.tile_pool(name="ppS", bufs=2, space="PSUM"))
        ppLG = ctx.enter_context(tc.tile_pool(name="ppLG", bufs=1, space="PSUM"))
        ppE = ctx.enter_context(tc.tile_pool(name="ppE", bufs=1, space="PSUM"))
        ppU = ctx.enter_context(tc.tile_pool(name="ppU", bufs=2, space="PSUM"))
        ppDN = ctx.enter_context(tc.tile_pool(name="ppDN", bufs=2, space="PSUM"))

        def cl(name, hdl, shape, dtype):
            t = const.tile(shape, dtype, tag=name)
            nc.sync.dma_start(t[:], hdl[:])
            return t

        ident = cl("ident", identI, [128, 128], dt.bfloat16)
        att1_t = cl("att1", att1bd, [128, 32], dt.bfloat16)
        bh1_t = const.tile([128, D1], dt.float32, tag="bh1")
        wl2_t = const.tile([128, 4 * D2], dt.bfloat16, tag="wl2")
        wr2_t = const.tile([128, 4 * D2], dt.bfloat16, tag="wr2")
        bxr2_t = const.tile([128, D2], dt.float32, tag="bxr2")
        wjk1_t = const.tile([128, 4 * 128], dt.bfloat16, tag="wjk1")
        xr1_t = const.tile([128, NW * D1], dt.bfloat16, tag="xr1t")

        def late_consts():
            nc.sync.dma_start(bh1_t[:], biash1[:])
            nc.sync.dma_start(wl2_t[:], Wl2p[:])
            nc.sync.dma_start(wr2_t[:], Wr2p[:])
            nc.sync.dma_start(bxr2_t[:], bxr2p[:])
            nc.sync.dma_start(wjk1_t[:], Wjk1p[:])

        def load_xr(w):
            nc.sync.dma_start(xr1_t[:, w * D1:(w + 1) * D1], xr1[w])

        def on_h(w, h):
            # xl2 = h@Wl2p ; xr2 = h@Wr2p + b ; jk01 = jk0 + h@Wjk1p
            p_xl2 = ppS.tile([128, D2], dt.float32, tag="s")
            p_xr2 = ppS.tile([128, D2], dt.float32, tag="s")
            p_jk = ppE.tile([128, 128], dt.float32, tag="ept")
            for g in range(4):
                tp = ppLG.tile([128, 128], dt.float32, tag="lg")
                nc.tensor.matmul(tp[:], lhsT=h[:, g * 128:(g + 1) * 128],
                                 rhs=ident[:], start=True, stop=True)
                hTs = sbuf.tile([128, 128], dt.bfloat16, tag="hT")
                nc.vector.tensor_copy(hTs[:], tp[:])
                nc.tensor.matmul(p_xl2[:], lhsT=hTs[:],
                                 rhs=wl2_t[:, g * D2:(g + 1) * D2],
                                 start=(g == 0), stop=(g == 3))
                nc.tensor.matmul(p_xr2[:], lhsT=hTs[:],
                                 rhs=wr2_t[:, g * D2:(g + 1) * D2],
                                 start=(g == 0), stop=(g == 3))
                nc.tensor.matmul(p_jk[:], lhsT=hTs[:],
                                 rhs=wjk1_t[:, g * 128:(g + 1) * 128],
                                 start=(g == 0), stop=(g == 3))
            o_xl2 = sbuf.tile([128, D2], dt.bfloat16, tag="oxl2")
            nc.any.tensor_copy(o_xl2[:], p_xl2[:])
            nc.gpsimd.dma_start(xl2_o[w * 128:(w + 1) * 128, :], o_xl2[:])
            o_xr2 = sbuf.tile([128, D2], dt.bfloat16, tag="oxr2")
            nc.vector.tensor_tensor(out=o_xr2[:], in0=p_xr2[:], in1=bxr2_t[:],
                                    op=ALU.add)
            nc.gpsimd.dma_start(xr2_o[w * 128:(w + 1) * 128, :], o_xr2[:])
            jk0_t = sbuf.tile([128, 128], dt.bfloat16, tag="jk0")
            nc.gpsimd.dma_start(jk0_t[:], jk0[w])
            o_jk = sbuf.tile([128, 128], dt.float32, tag="ojk")
            nc.vector.tensor_tensor(out=o_jk[:], in0=p_jk[:], in1=jk0_t[:],
                                    op=ALU.add)
            nc.gpsimd.dma_start(jk01_o[w * 128:(w + 1) * 128, :], o_jk[:])

        pools = dict(sbuf=sbuf, em=empool, goh=gohpool, ppS=ppS, ppLG=ppLG,
                     ppE=ppE, ppU=ppU, ppDN=ppDN)
        _emit_edge_pipeline(nc, pools, dict(
            D=D1, CH=C1, NSB=NSB, merged_dn=False, expe_on_act=True,
            emfm_dram=emfm, goh_dram=goh,
            xr_tile=xr1_t, att_tile=att1_t, biash_tile=bh1_t,
            ident=ident, on_h=on_h, late_consts=late_consts, load_xr=load_xr))

    nc.compile()
    return nc


def _build_launch_c(NSB):
    EPW = NSB * SBE
    nc = bacc.Bacc(None, target_bir_lowering=False)

    emfm = nc.dram_tensor("emfm", [NW, 128, NSB * (4 * (D2 + 8) + 2 * SBE)],
                          dt.bfloat16, kind="ExternalInput")
    goh = nc.dram_tensor("goh", [NW, 128, 2 * EPW], dt.bfloat16,
                         kind="ExternalInput")
    xr2 = nc.dram_tensor("xr2", [NW, 128, D2], dt.bfloat16,
                         kind="ExternalInput")
    jk01 = nc.dram_tensor("jk01", [NW, 128, 128], dt.float32,
                          kind="ExternalInput")
    att2bd = nc.dram_tensor("att2bd", [128, 16], dt.bfloat16,
                            kind="ExternalInput")
    biash2 = nc.dram_tensor("biash2", [128, D2], dt.float32,
                            kind="ExternalInput")
    identI = nc.dram_tensor("identI", [128, 128], dt.bfloat16,
                            kind="ExternalInput")
    Wjk2p = nc.dram_tensor("Wjk2p", [128, 2 * 128], dt.bfloat16,
                           kind="ExternalInput")
    bjk_r = nc.dram_tensor("bjk_r", [1, 128], dt.bfloat16,
                           kind="ExternalInput")
    ones1d = nc.dram_tensor("ones1", [1, 128], dt.bfloat16,
                            kind="ExternalInput")

    out_o = nc.dram_tensor("out_o", [NPAD, 128], dt.float32,
                           kind="ExternalOutput")

    with tile.TileContext(nc) as tc, ExitStack() as ctx:
        const = ctx.enter_context(tc.tile_pool(name="const", bufs=1))
        sbuf = ctx.enter_context(tc.tile_pool(name="sbuf", bufs=3))
        empool = ctx.enter_context(tc.tile_pool(name="em", bufs=3))
        gohpool = ctx.enter_context(tc.tile_pool(name="goh", bufs=2))
        ppS = ctx.enter_context(tc.tile_pool(name="ppS", bufs=2, space="PSUM"))
        ppLG = ctx.enter_context(tc.tile_pool(name="ppLG", bufs=1, space="PSUM"))
        ppE = ctx.enter_context(tc.tile_pool(name="ppE", bufs=1, space="PSUM"))
        ppU = ctx.enter_context(tc.tile_pool(name="ppU", bufs=2, space="PSUM"))

        def cl(name, hdl, shape, dtype):
            t = const.tile(shape, dtype, tag=name)
            nc.sync.dma_start(t[:], hdl[:])
            return t

        ident = cl("ident", identI, [128, 128], dt.bfloat16)
        att2_t = cl("att2", att2bd, [128, 16], dt.bfloat16)
        bh2_t = const.tile([128, D2], dt.float32, tag="bh2")
        wjk2_t = const.tile([128, 2 * 128], dt.bfloat16, tag="wjk2")
        bjkr_t = cl("bjkr", bjk_r, [1, 128], dt.bfloat16)
        ones1 = cl("ones1", ones1d, [1, 128], dt.bfloat16)
        xr2_t = const.tile([128, NW * D2], dt.bfloat16, tag="xr2t")

        def late_consts():
            nc.sync.dma_start(bh2_t[:], biash2[:])
            nc.sync.dma_start(wjk2_t[:], Wjk2p[:])

        def load_xr(w):
            nc.sync.dma_start(xr2_t[:, w * D2:(w + 1) * D2], xr2[w])

        def on_h(w, h):
            p_out = ppS.tile([128, 128], dt.float32, tag="s")
            nc.tensor.matmul(p_out[:], lhsT=ones1[:], rhs=bjkr_t[:],
                             start=True, stop=False)
            for g in range(2):
                tp = ppLG.tile([128, 128], dt.float32, tag="lg")
                nc.tensor.matmul(tp[:], lhsT=h[:, g * 128:(g + 1) * 128],
                                 rhs=ident[:], start=True, stop=True)
                hTs = sbuf.tile([128, 128], dt.bfloat16, tag="hT")
                nc.vector.tensor_copy(hTs[:], tp[:])
                nc.tensor.matmul(p_out[:], lhsT=hTs[:],
                                 rhs=wjk2_t[:, g * 128:(g + 1) * 128],
                                 start=False, stop=(g == 1))
            jk_t = sbuf.tile([128, 128], dt.float32, tag="jkt")
            nc.gpsimd.dma_start(jk_t[:], jk01[w])
            o_t = sbuf.tile([128, 128], dt.float32, tag="ot")
            nc.vector.tensor_tensor(out=o_t[:], in0=p_out[:], in1=jk_t[:],
                                    op=ALU.add)
            nc.gpsimd.dma_start(out_o[w * 128:(w + 1) * 128, :], o_t[:])

        pools = dict(sbuf=sbuf, em=empool, goh=gohpool, ppS=ppS, ppLG=ppLG,
                     ppE=ppE, ppU=ppU)
        _emit_edge_pipeline(nc, pools, dict(
            D=D2, CH=C2, NSB=NSB, merged_dn=True, expe_on_act=False,
            emfm_dram=emfm, goh_dram=goh,
            xr_tile=xr2_t, att_tile=att2_t, biash_tile=bh2_t,
            ident=ident, on_h=on_h, late_consts=late_consts, load_xr=load_xr))

    nc.compile()
    return nc


_PROGRAM_CACHE = {}


def kernel(x, edge_index, Wl1, bl1, Wr1, br1, att1, bias1,
           Wl2, bl2, Wr2, br2, att2, bias2, Wjk, bjk):
    global LAST_RESULTS
    LAST_RESULTS = []
    trace = bool(os.environ.get("GAT_TRACE"))

    x = _f32(x)
    Wl1, Wr1 = _f32(Wl1), _f32(Wr1)
    Wl2, Wr2 = _f32(Wl2), _f32(Wr2)
    Wjk = _f32(Wjk)
    NSB, srcs, gohs = _plan_edges(np.asarray(edge_index))

    if "A" not in _PROGRAM_CACHE:
        _PROGRAM_CACHE["A"] = _build_launch_a()
    if ("B", NSB) not in _PROGRAM_CACHE:
        _PROGRAM_CACHE[("B", NSB)] = _build_launch_b(NSB)
    if ("C", NSB) not in _PROGRAM_CACHE:
        _PROGRAM_CACHE[("C", NSB)] = _build_launch_c(NSB)

    ident = np.eye(128, dtype=np.float32)

    # ---------------- launch A: per-node transforms ----------------
    common_a = dict(
        Wl1p=_bf(Wl1[:, PERM1]),
        Wr1p=_bf(Wr1[:, PERM1]),
        bxr1p=_f32(np.tile((np.asarray(bl1) + np.asarray(br1))[PERM1][None, :],
                           (128, 1))),
        Wjk0=_bf(Wjk[:128]),
    )
    in_maps_a = []
    for c in range(NCORES):
        xo = np.zeros((128, NPAD), np.float32)
        xo[:, :NPC] = x[c * NPC:(c + 1) * NPC].T
        in_maps_a.append(dict(common_a, x_ownT=_bf(xo)))

    res_a = run_bass_kernel_spmd(_PROGRAM_CACHE["A"], in_maps_a,
                                 core_ids=list(range(NCORES)), trace=trace)
    LAST_RESULTS.append(res_a)

    # ---------------- host routing for layer 1 ----------------
    ao = [np.asarray(res_a.results[c]["ao"]) for c in range(NCORES)]
    xl1_all = np.concatenate([a[:NPC, :D1] for a in ao], axis=0)
    emfm1 = _route_edges(xl1_all, srcs, NSB, with_ones=False)

    common_b = dict(
        att1bd=_bf(_att_bd(np.asarray(att1), D1)),
        biash1=_f32(np.tile((np.asarray(bl1) + np.asarray(bias1))[PERM1][None, :],
                            (128, 1))),
        identI=_bf(ident),
        Wl2p=_bf(Wl2[PERM1][:, PERM2].reshape(4, 128, D2)
                 .transpose(1, 0, 2).reshape(128, 4 * D2)),
        Wr2p=_bf(Wr2[PERM1][:, PERM2].reshape(4, 128, D2)
                 .transpose(1, 0, 2).reshape(128, 4 * D2)),
        bxr2p=_f32(np.tile((np.asarray(bl2) + np.asarray(br2))[PERM2][None, :],
                           (128, 1))),
        Wjk1p=_bf(Wjk[128:128 + D1][PERM1].reshape(4, 128, 128)
                  .transpose(1, 0, 2).reshape(128, 4 * 128)),
    )
    in_maps_b = []
    for c in range(NCORES):
        in_maps_b.append(dict(
            common_b,
            emfm=emfm1[c],
            goh=gohs[c],
            xr1=np.ascontiguousarray(ao[c][:, D1:2 * D1].reshape(NW, 128, D1)),
            jk0=np.ascontiguousarray(ao[c][:, 2 * D1:].reshape(NW, 128, 128)),
        ))

    res_b = run_bass_kernel_spmd(_PROGRAM_CACHE[("B", NSB)], in_maps_b,
                                 core_ids=list(range(NCORES)), trace=trace)
    LAST_RESULTS.append(res_b)

    # ---------------- host routing for layer 2 ----------------
    xl2_all = np.concatenate(
        [np.asarray(res_b.results[c]["xl2_o"])[:NPC] for c in range(NCORES)],
        axis=0)                                   # [N, 256] bf16, interleaved
    emfm2 = _route_edges(xl2_all, srcs, NSB, with_ones=True)

    common_c = dict(
        att2bd=_bf(_att_bd(np.asarray(att2), D2)),
        biash2=_f32(np.tile((np.asarray(bl2) + np.asarray(bias2))[PERM2][None, :],
                            (128, 1))),
        identI=_bf(ident),
        Wjk2p=_bf(Wjk[128 + D1:][PERM2].reshape(2, 128, 128)
                  .transpose(1, 0, 2).reshape(128, 2 * 128)),
        bjk_r=_bf(np.asarray(bjk)[None, :]),
        ones1=_bf(np.ones((1, 128), np.float32)),
    )
    in_maps_c = []
    for c in range(NCORES):
        in_maps_c.append(dict(
            common_c,
            emfm=emfm2[c],
            goh=gohs[c],
            xr2=np.asarray(res_b.results[c]["xr2_o"]).reshape(NW, 128, D2),
            jk01=_f32(np.asarray(res_b.results[c]["jk01_o"])
                      .reshape(NW, 128, 128)),
        ))

    res_c = run_bass_kernel_spmd(_PROGRAM_CACHE[("C", NSB)], in_maps_c,
                                 core_ids=list(range(NCORES)), trace=trace)
    LAST_RESULTS.append(res_c)

    out = np.concatenate(
        [np.asarray(res_c.results[c]["out_o"])[:NPC] for c in range(NCORES)],
        axis=0)
    return np.ascontiguousarray(out, dtype=np.float32)
